# revision 10
# baseline (speedup 1.0000x reference)
"""Trainium2 Bass kernel for nn_DeepFusionLayers (topk_masking).

Sharding: data-parallel over H (8 cores x 24 rows); halo handled by
recompute from host-prepared overlapping slabs (rows delta in [-1,25)).
3x3 convs = 9 shifted matmuls accumulating in PSUM (C=96 on partitions,
W padded to 194 so shifts never wrap rows). The channel attention runs
on 8x8-maxpooled features: pooled locally, AllGathered (small), then
every core computes the tiny attention redundantly; top-k thresholds via
progressive Max8/match_replace extraction. The 8x nearest upsample is
fused into the proj matmul via step-0 access patterns, reading a
per-core 5-row window of the attention output selected with a dynamic
(partition-id-derived) DRAM slice. EAF BatchNorm uses global stats via
tiny AllGathers; the bn affine transform is folded into the sim matmul
(3 accumulating matmuls over raw conv outputs). bf16 compute, fp32 PSUM.
"""
import numpy as np
import ml_dtypes

import concourse.bass as bass
import concourse.bacc as bacc
import concourse.tile as tile
from concourse import mybir
from concourse.bass_utils import run_bass_kernel_spmd
from concourse.masks import make_identity

F32 = mybir.dt.float32
BF16 = mybir.dt.bfloat16
AF = mybir.ActivationFunctionType
OP = mybir.AluOpType
BF = ml_dtypes.bfloat16

# geometry
B, C, H, W = 2, 96, 192, 192
HEADS, DS, L = 2, 8, 2
NCORE = 8
RO = 24            # owned rows per core
RC = 26            # chain rows: delta in [-1, 25)
WP = W + 2         # padded width (194)
PH, PW = 24, 24    # pooled image
PHL = 3            # pooled rows owned per core
PP2 = PW + 2       # padded pooled width (26)
NP576 = PH * PW
NPOS = B * H * W   # batchnorm population per channel
SENT = -1e30       # topk extraction sentinel
TOPK_KS = [24, 32, 36, 38]
SL3 = (RO - 1) * WP + W  # 3x3 conv flat stream length per sample (4654)

# params column map (f32 tensor "pp" [96, PPN])
_pcols = []
PCOL = {}
def _pc(name, n=1):
    PCOL[name] = len(_pcols)
    _pcols.extend([name] * n)
_pc("mkw_b", L)            # + l
_pc("qkv_b", L * 3)        # + l*3 + m
_pc("dw_bq", L)            # + l
_pc("dw_bk", L)            # + l
_pc("dw_bv", L * 2)        # + l*2 + h  (rows 0..47)
_pc("proj_b", L)           # + l
_pc("cab_b", 1)
_pc("temp", L)             # + l ; row pattern temp[l, p//48]
_pc("aw", L * 4)           # + l*4 + j
_pc("topmask", 1)          # per-core
_pc("botmask", 1)          # per-core
PPN = len(_pcols)

_CACHE = {}


def _chunks(total, maxn=512):
    return [(c, min(maxn, total - c)) for c in range(0, total, maxn)]


def _flat(t):
    ap = t[:]
    nd = len(ap.shape)
    if nd == 2:
        return ap
    names = " ".join(chr(ord("a") + i) for i in range(nd - 1))
    return ap.rearrange(f"p {names} -> p ({names})")


def _rap(t, off, dims):
    """raw AP over pool tile / free-dim offset (elements) + free dims list."""
    return bass.AP(tensor=t.tensor, offset=t.offset + off,
                   ap=[list(t.ap[0])] + [list(d) for d in dims])


def _build(debug=False):
    key = ("mod", debug)
    if key in _CACHE:
        return _CACHE[key]
    nc = bacc.Bacc(None, num_devices=NCORE)

    # ---------------- dram parameters ----------------
    x0s = nc.declare_dram_parameter("x0s", [C, B, RC, WP], BF16, isOutput=False)
    hfs = nc.declare_dram_parameter("hfs", [C, B, RC, W], BF16, isOutput=False)
    lfs = nc.declare_dram_parameter("lfs", [C, B, RC, W], BF16, isOutput=False)
    w_mkw = nc.declare_dram_parameter("w_mkw", [C, L, 9, C], BF16, isOutput=False)
    w_qkv = nc.declare_dram_parameter("w_qkv", [C, L, 3 * C], BF16, isOutput=False)
    w_dwq = nc.declare_dram_parameter("w_dwq", [C, L, 9, C], BF16, isOutput=False)
    w_dwk = nc.declare_dram_parameter("w_dwk", [C, L, 9, C], BF16, isOutput=False)
    w_dwv = nc.declare_dram_parameter("w_dwv", [C, L, 9, 2, 48], BF16, isOutput=False)
    w_proj = nc.declare_dram_parameter("w_proj", [48, L, 2, C], BF16, isOutput=False)
    w_eaf = nc.declare_dram_parameter("w_eaf", [C, L, 2, 2, 48], BF16, isOutput=False)
    w_cab = nc.declare_dram_parameter("w_cab", [C, 3, 9, C], BF16, isOutput=False)
    ppar = nc.declare_dram_parameter("pp", [C, PPN], F32, isOutput=False)
    out_p = nc.declare_dram_parameter("out", [C, B, RO, W], F32, isOutput=True)
    dbg = {}
    if debug:
        def _dbg(name, shape, dt=BF16):
            dbg[name] = nc.declare_dram_parameter(name, shape, dt, isOutput=True)
        _dbg("dbg_mkw0", [C, B, RO, WP])
        _dbg("dbg_pq0", [C, B, PP2, PP2])
        _dbg("dbg_q0", [C, B, NP576])
        _dbg("dbg_v0", [48, B, NP576])
        _dbg("dbg_attn0", [48, 4, 48], F32)
        _dbg("dbg_acomb0", [48, 4, 48], F32)
        _dbg("dbg_os0", [48, 2, B, NP576])
        _dbg("dbg_xdt0", [C, B, RC, W])
        _dbg("dbg_sim0", [C, B, RC, W])
        _dbg("dbg_xk0", [48, B, RC, W])
        _dbg("dbg_x1", [C, B, RC, WP])
        _dbg("dbg_x2", [C, B, RC, WP])

    with tile.TileContext(nc) as tc:
        import contextlib
        stack = contextlib.ExitStack()
        wp = stack.enter_context(tc.tile_pool(name="wp", bufs=1))
        slab = stack.enter_context(tc.tile_pool(name="slab", bufs=1))
        scr = stack.enter_context(tc.tile_pool(name="scr", bufs=3))
        sm = stack.enter_context(tc.tile_pool(name="sm", bufs=1))
        at = stack.enter_context(tc.tile_pool(name="at", bufs=2))
        ps = stack.enter_context(tc.tile_pool(name="ps", bufs=4, space="PSUM"))
        ps2 = stack.enter_context(tc.tile_pool(name="ps2", bufs=3, space="PSUM"))
        dram = stack.enter_context(tc.tile_pool(name="dram", bufs=1, space="DRAM"))

        # ---------------- load weights / params ----------------
        def _load(param, shape, nm, dtype=BF16):
            t = wp.tile(shape, dtype, tag=nm, name=nm)
            nc.sync.dma_start(out=t[:], in_=param[:])
            return t

        wmkw = _load(w_mkw, [C, L, 9, C], "wmkw")
        wqkv = _load(w_qkv, [C, L, 3 * C], "wqkv")
        wdwq = _load(w_dwq, [C, L, 9, C], "wdwq")
        wdwk = _load(w_dwk, [C, L, 9, C], "wdwk")
        wdwv = _load(w_dwv, [C, L, 9, 2, 48], "wdwv")
        wproj = _load(w_proj, [48, L, 2, C], "wproj")
        weaf = _load(w_eaf, [C, L, 2, 2, 48], "weaf")
        pp = _load(ppar, [C, PPN], "ppt", F32)

        def ppc(name, idx=0, p0=0, pn=C):
            c = PCOL[name] + idx
            return pp[p0:p0 + pn, c:c + 1]

        ident = wp.tile([128, 128], BF16)
        make_identity(nc, ident[:])
        ones48 = wp.tile([48, C], BF16)
        nc.vector.memset(ones48[:], 1.0)

        # persistent slabs
        x1t = slab.tile([C, B, RC, WP], BF16, tag="x1t")
        x2t = slab.tile([C, B, RC, WP], BF16, tag="x2t")
        xw = slab.tile([C, B, RC, W], BF16, tag="xw")  # x_dt / x_eh work slab

        # out_small DRAM bounces (pad rows 0 and 25), one per head
        osb = [dram.tile([48, B, PH + 2, PW], BF16, tag=f"osb{h}", name=f"osb{h}") for h in range(HEADS)]
        zpad = sm.tile([48, B, 2, PW], BF16, tag="zpad")
        nc.vector.memset(zpad[:], 0.0)
        for h in range(HEADS):
            nc.sync.dma_start(out=osb[h][:, :, 0:1, :], in_=zpad[:, :, 0:1, :])
            nc.sync.dma_start(out=osb[h][:, :, PH + 1:PH + 2, :], in_=zpad[:, :, 1:2, :])

        # per-core pooled-row window offset register: 3 * partition_id
        with nc.sync.register("goff") as goff:
            nc.sync.reg_load(goff, nc.partition_id_tensor[0:1, 0:1])
            nc.sync.reg_alu(goff, goff, PHL, OP.mult)
            offv = nc.sync.snap(goff)

        rg = [list(range(NCORE))]

        def conv3x3(pairs, M, chunk_list, epilogue):
            """pairs: [(lhsT3 [*,9,C], src_tile [*,B,RC,WP])]; writes per-b
            chunks of the flat output stream (start (row1,col1))."""
            for b in range(B):
                taps = []
                for (lhsT3, src_t) in pairs:
                    sf = _flat(src_t)
                    for t9 in range(9):
                        dy, dx = t9 // 3, t9 % 3
                        off = b * (RC * WP) + dy * WP + dx
                        taps.append(
                            (lhsT3[:, t9, :],
                             (lambda sf, off: lambda c0, n: sf[:, off + c0:off + c0 + n])(sf, off)))
                for (c0, n) in chunk_list:
                    pt = ps.tile([C, 512], F32, tag="cps")
                    for t, (lh, rfn) in enumerate(taps):
                        nc.tensor.matmul(pt[:, :n], lh, rfn(c0, n),
                                         start=(t == 0), stop=(t == len(taps) - 1))
                    epilogue(b, c0, n, pt[:, :n])

        # =====================================================
        for l in range(L):
            if l == 0:
                x0t = scr.tile([C, B, RC, WP], BF16, tag="scr")
                nc.sync.dma_start(out=x0t[:], in_=x0s[:])
                src = x0t
            else:
                src = x1t

            # ---------------- MKW conv (3x3 96->96, owned rows) ---------
            mkwout = scr.tile([C, B, RO, WP], BF16, tag="scr")
            mkf = _flat(mkwout)

            def mkw_ep(b, c0, n, pt):
                nc.scalar.activation(mkf[:, b * (RO * WP) + 1 + c0: b * (RO * WP) + 1 + c0 + n],
                                     pt, AF.Identity, bias=ppc("mkw_b", l))

            conv3x3([(wmkw[:, l], src)], C, _chunks(SL3), mkw_ep)
            if debug and l == 0:
                nc.sync.dma_start(out=dbg["dbg_mkw0"][:], in_=mkwout[:])

            # ---------------- qkv conv (1x1 96->288) + 8x8 maxpool ------
            ag1_in = dram.tile([3, C, B, PHL, PW], BF16)
            ag1_out = dram.tile([3 * NCORE, C, B, PHL, PW], BF16, addr_space="Shared")
            for m in range(3):
                qkvf = scr.tile([C, B, RO, W], BF16, tag="scr")
                for b in range(B):
                    for r in range(0, RO, 2):
                        pt = ps.tile([C, 512], F32, tag="cps")
                        nc.tensor.matmul(pt[:, :384], wqkv[:, l, m * C:(m + 1) * C],
                                         mkwout[:, b, r:r + 2, 1:1 + W],
                                         start=True, stop=True)
                        nc.scalar.copy(qkvf[:, b, r:r + 2, :],
                                       pt[:, :384].rearrange("p (a b) -> p a b", a=2))
                # pool: W-max, then H-max, then +bias -> bf16
                pwt = sm.tile([C, B, PH, PW], BF16, tag="pwt")
                pmt = sm.tile([C, B, PHL, PW], BF16, tag="pmt")
                for b in range(B):
                    nc.vector.tensor_reduce(
                        out=pwt[:, b],
                        in_=qkvf[:, b].rearrange("p r (w e) -> p r w e", e=DS),
                        axis=mybir.AxisListType.X, op=OP.max)
                    nc.vector.tensor_reduce(
                        out=pmt[:, b],
                        in_=_rap(pwt, b * PH * PW, [[DS * PW, PHL], [1, PW], [PW, DS]]),
                        axis=mybir.AxisListType.X, op=OP.max)
                contrib = sm.tile([C, B, PHL, PW], BF16, tag="contrib")
                nc.scalar.activation(_flat(contrib), _flat(pmt), AF.Identity,
                                     bias=ppc("qkv_b", l * 3 + m))
                nc.sync.dma_start(out=ag1_in[m], in_=contrib[:])
            nc.gpsimd.collective_compute(
                "AllGather", OP.bypass, replica_groups=rg,
                ins=[ag1_in[:].opt()], outs=[ag1_out[:].opt()])

            # readback into padded pooled tiles (via compact staging)
            pooled = []
            poolc = sm.tile([C, B, PH, PW], BF16, tag="poolc")
            for m in range(3):
                pt_ = sm.tile([C, B, PP2, PP2], BF16, tag=f"pool{m}", name=f"pool{m}")
                nc.vector.memset(pt_[:], 0.0)
                for b in range(B):
                    src_ap = bass.AP(
                        tensor=ag1_out.tensor,
                        offset=ag1_out.offset + m * C * (B * PHL * PW) + b * (PHL * PW),
                        ap=[[B * PHL * PW, C], [3 * C * B * PHL * PW, NCORE],
                            [1, PHL * PW]])
                    dst_ap = _rap(poolc, b * PH * PW, [[PHL * PW, NCORE], [1, PHL * PW]])
                    nc.sync.dma_start(out=dst_ap, in_=src_ap)
                    nc.vector.tensor_copy(
                        _rap(pt_, b * PP2 * PP2 + PP2 + 1, [[PP2, PH], [1, PW]]),
                        _rap(poolc, b * PH * PW, [[PW, PH], [1, PW]]))
                pooled.append(pt_)
            if debug and l == 0:
                nc.sync.dma_start(out=dbg["dbg_pq0"][:], in_=pooled[0][:])

            # ---------------- depthwise 3x3 conv on pooled --------------
            DWCH = [(0, 11 * PP2 + PW), (12 * PP2, 11 * PP2 + PW)]  # rows 1-12, 13-24
            qsb = sm.tile([C, B, NP576], BF16, tag="qsb")
            ksb = sm.tile([C, B, NP576], BF16, tag="ksb")
            vsb = [sm.tile([48, B, NP576], BF16, tag=f"vsb{h}", name=f"vsb{h}") for h in range(HEADS)]

            def dw_run(lhsT9, src_t, M, dst, dst_p0, bias_ap):
                sf = _flat(src_t)
                for b in range(B):
                    for ci, (s0, n) in enumerate(DWCH):
                        pt = ps.tile([M, 512], F32, tag="cps")
                        for t9 in range(9):
                            dy, dx = t9 // 3, t9 % 3
                            off = b * (PP2 * PP2) + dy * PP2 + dx + s0
                            nc.tensor.matmul(pt[:M, :n], lhsT9[:, t9, :],
                                             sf[:, off:off + n],
                                             start=(t9 == 0), stop=(t9 == 8))
                        src_ap = bass.AP(tensor=pt.tensor, offset=pt.offset,
                                         ap=[[pt.ap[0][0], M], [PP2, 12], [1, PW]])
                        dst_ap = bass.AP(
                            tensor=dst.tensor,
                            offset=dst.offset + b * NP576 + ci * 288,
                            ap=[[dst.ap[0][0], M], [PW, 12], [1, PW]])
                        nc.scalar.activation(dst_ap, src_ap, AF.Identity, bias=bias_ap)

            dw_run(wdwq[:, l], pooled[0], C, qsb, 0, ppc("dw_bq", l))
            dw_run(wdwk[:, l], pooled[1], C, ksb, 0, ppc("dw_bk", l))
            for h in range(HEADS):
                dw_run(wdwv[:, l, :, h, :], pooled[2], 48, vsb[h], 0,
                       ppc("dw_bv", l * 2 + h, 0, 48))
            if debug and l == 0:
                nc.sync.dma_start(out=dbg["dbg_q0"][:], in_=qsb[:])
                nc.sync.dma_start(out=dbg["dbg_v0"][:], in_=vsb[0][:])

            # ---------------- normalize q, k (rows of [48,576]) ---------
            sq = sm.tile([C, NP576], F32, tag="sq")
            nrm = sm.tile([C, B, 4], F32, tag="nrm")  # qn kn qscale(kept in 0/1)
            for b in range(B):
                nc.scalar.activation(sq[:], qsb[:, b], AF.Square, accum_out=nrm[:, b, 0:1])
                nc.scalar.activation(sq[:], ksb[:, b], AF.Square, accum_out=nrm[:, b, 1:2])
                nc.scalar.activation(nrm[:, b, 0:2], nrm[:, b, 0:2], AF.Sqrt)
                nc.vector.tensor_scalar_max(nrm[:, b, 0:2], nrm[:, b, 0:2], 1e-12)
                nc.vector.reciprocal(nrm[:, b, 0:2], nrm[:, b, 0:2])
                nc.vector.tensor_scalar(nrm[:, b, 2:3], nrm[:, b, 0:1],
                                        ppc("temp", l), None, op0=OP.mult)
                nc.vector.tensor_scalar(qsb[:, b], qsb[:, b], nrm[:, b, 2:3], None, op0=OP.mult)
                nc.vector.tensor_scalar(ksb[:, b], ksb[:, b], nrm[:, b, 1:2], None, op0=OP.mult)

            # ---------------- transpose q, k ----------------------------
            KCH = [(0, 128), (128, 128), (256, 128), (384, 128), (512, 64)]
            qT = sm.tile([128, B, 5, C], BF16, tag="qT")
            kT = sm.tile([128, B, 5, C], BF16, tag="kT")
            for (src_t, dst) in ((qsb, qT), (ksb, kT)):
                for b in range(B):
                    for ci, (c0, cw) in enumerate(KCH):
                        tp = ps2.tile([128, C], BF16, tag="p2")
                        nc.tensor.transpose(tp[:cw, :], src_t[:, b, c0:c0 + cw],
                                            ident[0:C, 0:C])
                        nc.scalar.copy(dst[:cw, b, ci, :], tp[:cw, :])

            # ---------------- per-pair attention + topk -----------------
            osm = [at.tile([48, B, NP576], BF16, tag=f"osm{h}", name=f"osm{h}") for h in range(HEADS)]
            for b in range(B):
                for h in range(HEADS):
                    pair = b * HEADS + h
                    atp = ps2.tile([48, 48], F32, tag="p2")
                    for ci, (c0, cw) in enumerate(KCH):
                        nc.tensor.matmul(atp[:], qT[:cw, b, ci, h * 48:(h + 1) * 48],
                                         kT[:cw, b, ci, h * 48:(h + 1) * 48],
                                         start=(ci == 0), stop=(ci == 4))
                    A = at.tile([48, 48], F32, tag="A")
                    nc.vector.tensor_copy(A[:], atp[:])
                    if debug and l == 0:
                        nc.sync.dma_start(out=dbg["dbg_attn0"][:, pair, :], in_=A[:])
                    rm = at.tile([48, 2], F32, tag="rm")
                    nc.vector.tensor_reduce(out=rm[:, 0:1], in_=A[:],
                                            axis=mybir.AxisListType.X, op=OP.max)
                    nc.vector.tensor_scalar_mul(rm[:, 1:2], rm[:, 0:1], -1.0)
                    E = at.tile([48, 48], F32, tag="E")
                    nc.scalar.activation(E[:], A[:], AF.Exp, bias=rm[:, 1:2])
                    wk2 = [at.tile([48, 48], F32, tag="wka", name="wka"),
                           at.tile([48, 48], F32, tag="wkb", name="wkb")]
                    nc.vector.tensor_copy(wk2[0][:], A[:])
                    cur = 0
                    mx = at.tile([48, 8], F32, tag="mx")
                    acc = at.tile([48, 48], F32, tag="acc")
                    em = at.tile([48, 48], F32, tag="em")
                    sk = at.tile([48, 2], F32, tag="sk")
                    wk = at.tile([48, 1], F32, tag="wk")
                    prev, ik = 0, 0
                    for kk in [8, 16, 24, 32, 36, 38]:
                        nfind = kk - prev
                        nc.vector.max(mx[:], wk2[cur][:])
                        if nfind < 8:
                            nc.vector.memset(mx[:, nfind:], SENT)
                        nc.vector.match_replace(out=wk2[1 - cur][:], in_to_replace=mx[:],
                                                in_values=wk2[cur][:], imm_value=SENT)
                        cur = 1 - cur
                        prev = kk
                        if kk in TOPK_KS:
                            nc.vector.tensor_scalar(em[:], wk2[cur][:], SENT, None,
                                                    op0=OP.is_equal)
                            nc.vector.tensor_mul(em[:], em[:], E[:])
                            nc.vector.tensor_reduce(out=sk[:, 0:1], in_=em[:],
                                                    axis=mybir.AxisListType.X, op=OP.add)
                            nc.vector.reciprocal(sk[:, 1:2], sk[:, 0:1])
                            nc.vector.tensor_scalar(wk[:], sk[:, 1:2],
                                                    ppc("aw", l * 4 + ik, 0, 48),
                                                    None, op0=OP.mult)
                            if ik == 0:
                                nc.vector.tensor_scalar(acc[:], em[:], wk[:], None, op0=OP.mult)
                            else:
                                nc.vector.scalar_tensor_tensor(
                                    out=acc[:], in0=em[:], scalar=wk[:], in1=acc[:],
                                    op0=OP.mult, op1=OP.add)
                            ik += 1
                    acb = at.tile([48, 48], BF16, tag="acb")
                    nc.vector.tensor_copy(acb[:], acc[:])
                    if debug and l == 0:
                        nc.sync.dma_start(out=dbg["dbg_acomb0"][:, pair, :], in_=acc[:])
                    att = ps2.tile([48, 48], BF16, tag="p2")
                    nc.tensor.transpose(att[:], acb[:], ident[0:48, 0:48])
                    avw = at.tile([48, 48], BF16, tag="avw")
                    nc.scalar.copy(avw[:], att[:])
                    for half in range(2):
                        avp = ps2.tile([48, 288], F32, tag="p2")
                        nc.tensor.matmul(avp[:], avw[:],
                                         vsb[h][:, b, half * 288:(half + 1) * 288],
                                         start=True, stop=True)
                        nc.scalar.activation(osm[h][:, b, half * 288:(half + 1) * 288],
                                             avp[:], AF.Gelu)
            if debug and l == 0:
                for h in range(HEADS):
                    nc.sync.dma_start(out=dbg["dbg_os0"][:, h], in_=osm[h][:])

            # ---------------- window + upsample-fused proj --------------
            for h in range(HEADS):
                nc.sync.dma_start(out=osb[h][:, :, 1:1 + PH, :],
                                  in_=osm[h][:].rearrange("p b (r w) -> p b r w", r=PH))
            wnd = [sm.tile([48, B, 5, PW], BF16, tag=f"wnd{h}", name=f"wnd{h}") for h in range(HEADS)]
            for h in range(HEADS):
                nc.sync.dma_start(out=wnd[h][:], in_=osb[h][:, :, bass.ds(offv, 5), :])
            for b in range(B):
                for r in range(RC):
                    wr = (r - 1) // DS + 1
                    pt = ps.tile([C, 512], F32, tag="cps")
                    for h in range(HEADS):
                        rhs = bass.AP(
                            tensor=wnd[h].tensor,
                            offset=wnd[h].offset + (b * 5 + wr) * PW,
                            ap=[[wnd[h].ap[0][0], 48], [1, PW], [0, DS]])
                        nc.tensor.matmul(pt[:, :W], wproj[:, l, h, :], rhs,
                                         start=(h == 0), stop=(h == 1))
                    nc.scalar.activation(xw[:, b, r, :], pt[:, :W], AF.Identity,
                                         bias=ppc("proj_b", l))
            if debug and l == 0:
                nc.sync.dma_start(out=dbg["dbg_xdt0"][:], in_=xw[:])

            # ---------------- EAF (h then l) ----------------------------
            def eaf(y_param, e_idx, out_write, dbg_keys=None):
                ysl = scr.tile([C, B, RC, W], BF16, tag="scr")
                nc.sync.dma_start(out=ysl[:], in_=y_param[:])
                xk = scr.tile([48, B, RC, W], BF16, tag="scr")
                yq = scr.tile([48, B, RC, W], BF16, tag="scr")
                for (w_i, src_t, dst) in ((0, xw, xk), (1, ysl, yq)):
                    lhsT = weaf[:, l, e_idx, w_i, :]
                    for b in range(B):
                        for r in range(0, RC, 2):
                            pt = ps.tile([48, 512], F32, tag="cps")
                            nc.tensor.matmul(pt[:48, :384], lhsT, src_t[:, b, r:r + 2, :],
                                             start=True, stop=True)
                            nc.scalar.copy(dst[:, b, r:r + 2, :],
                                           pt[:48, :384].rearrange("p (a b) -> p a b", a=2))
                # local stats over owned rows (slab rows 1..24)
                stt = sm.tile([48, B, 12, 6], F32, tag="stt")
                mv = sm.tile([48, 2, 2], F32, tag="mv")
                agf = sm.tile([48, 4], F32, tag="agf")
                NL = float(B * RO * W)
                for ti, src_t in enumerate((xk, yq)):
                    for b in range(B):
                        for i in range(12):
                            nc.vector.bn_stats(
                                out=stt[:, b, i, :],
                                in_=src_t[:, b, 1 + 2 * i:3 + 2 * i, :].rearrange("p a b -> p (a b)"))
                    nc.vector.bn_aggr(out=mv[:, ti],
                                      in_=stt[:].rearrange("p a b c -> p (a b) c"))
                    nc.vector.tensor_scalar(agf[:, 2 * ti:2 * ti + 1], mv[:, ti, 0:1],
                                            NL, None, op0=OP.mult)
                    nc.vector.tensor_mul(agf[:, 2 * ti + 1:2 * ti + 2],
                                         mv[:, ti, 0:1], mv[:, ti, 0:1])
                    nc.vector.tensor_add(agf[:, 2 * ti + 1:2 * ti + 2],
                                         agf[:, 2 * ti + 1:2 * ti + 2], mv[:, ti, 1:2])
                    nc.vector.tensor_scalar(agf[:, 2 * ti + 1:2 * ti + 2],
                                            agf[:, 2 * ti + 1:2 * ti + 2], NL, None, op0=OP.mult)
                ags_in = dram.tile([48, 4], F32)
                ags_out = dram.tile([48 * NCORE, 4], F32, addr_space="Shared")
                nc.sync.dma_start(out=ags_in[:], in_=agf[:])
                nc.gpsimd.collective_compute(
                    "AllGather", OP.bypass, replica_groups=rg,
                    ins=[ags_in[:].opt()], outs=[ags_out[:].opt()])
                rb = sm.tile([48, 4, NCORE], F32, tag="rb")
                nc.sync.dma_start(out=rb[:], in_=bass.AP(
                    tensor=ags_out.tensor, offset=ags_out.offset,
                    ap=[[4, 48], [1, 4], [48 * 4, NCORE]]))
                gs = sm.tile([48, 4], F32, tag="gs")
                nc.vector.tensor_reduce(out=gs[:], in_=rb[:],
                                        axis=mybir.AxisListType.X, op=OP.add)
                # s,t per channel: mean=S/N; var=Q/N-mean^2; s=1/sqrt(var+eps); t=-mean*s
                stv = sm.tile([48, 8], F32, tag="stv")  # sx tx sy ty | ss st ts tt
                for ti in range(2):
                    nc.vector.tensor_scalar(stv[:, 4:5], gs[:, 2 * ti:2 * ti + 1],
                                            1.0 / NPOS, None, op0=OP.mult)
                    nc.vector.tensor_scalar(stv[:, 5:6], gs[:, 2 * ti + 1:2 * ti + 2],
                                            1.0 / NPOS, None, op0=OP.mult)
                    nc.vector.tensor_mul(stv[:, 6:7], stv[:, 4:5], stv[:, 4:5])
                    nc.vector.tensor_sub(stv[:, 5:6], stv[:, 5:6], stv[:, 6:7])
                    nc.vector.tensor_scalar(stv[:, 5:6], stv[:, 5:6], 1e-5, None, op0=OP.add)
                    nc.scalar.activation(stv[:, 5:6], stv[:, 5:6], AF.Sqrt)
                    nc.vector.reciprocal(stv[:, 2 * ti:2 * ti + 1], stv[:, 5:6])
                    nc.vector.tensor_mul(stv[:, 2 * ti + 1:2 * ti + 2], stv[:, 4:5],
                                         stv[:, 2 * ti:2 * ti + 1])
                    nc.vector.tensor_scalar_mul(stv[:, 2 * ti + 1:2 * ti + 2],
                                                stv[:, 2 * ti + 1:2 * ti + 2], -1.0)
                nc.vector.tensor_mul(stv[:, 4:5], stv[:, 0:1], stv[:, 2:3])  # ss
                nc.vector.tensor_mul(stv[:, 5:6], stv[:, 0:1], stv[:, 3:4])  # st
                nc.vector.tensor_mul(stv[:, 6:7], stv[:, 1:2], stv[:, 2:3])  # ts
                nc.vector.tensor_mul(stv[:, 7:8], stv[:, 1:2], stv[:, 3:4])  # tt
                vss = sm.tile([48, C], BF16, tag="vss")
                vst = sm.tile([48, C], BF16, tag="vst")
                vts = sm.tile([48, C], BF16, tag="vts")
                nc.vector.tensor_scalar(vss[:], ones48[:], stv[:, 4:5], None, op0=OP.mult)
                nc.vector.tensor_scalar(vst[:], ones48[:], stv[:, 5:6], None, op0=OP.mult)
                nc.vector.tensor_scalar(vts[:], ones48[:], stv[:, 6:7], None, op0=OP.mult)
                vtt = sm.tile([48, 1], BF16, tag="vtt")
                nc.vector.tensor_copy(vtt[:], stv[:, 7:8])
                cb = ps2.tile([C, 1], F32, tag="p2")
                nc.tensor.matmul(cb[:], ones48[:], vtt[:], start=True, stop=True)
                cbs = sm.tile([C, 1], F32, tag="cbs")
                nc.vector.tensor_copy(cbs[:], cb[:])
                # sim + blend, chunked
                xkf, yqf, xwf, ysf = _flat(xk), _flat(yq), _flat(xw), _flat(ysl)
                for (c0, n) in _chunks(B * RC * W):
                    pch = sm.tile([48, 512], BF16, tag="pch")
                    nc.vector.tensor_mul(pch[:, :n], xkf[:, c0:c0 + n], yqf[:, c0:c0 + n])
                    spt = ps.tile([C, 512], F32, tag="cps")
                    nc.tensor.matmul(spt[:, :n], vss[:], pch[:48, :n], start=True, stop=False)
                    nc.tensor.matmul(spt[:, :n], vst[:], xkf[:, c0:c0 + n], start=False, stop=False)
                    nc.tensor.matmul(spt[:, :n], vts[:], yqf[:, c0:c0 + n], start=False, stop=True)
                    sim = sm.tile([C, 512], BF16, tag="sim")
                    nc.scalar.activation(sim[:, :n], spt[:, :n], AF.Sigmoid, bias=cbs[:, 0:1])
                    if dbg_keys is not None:
                        nc.sync.dma_start(out=_flat(dbg[dbg_keys[1]])[:, c0:c0 + n],
                                          in_=sim[:, :n])
                    nc.vector.tensor_sub(xwf[:, c0:c0 + n], xwf[:, c0:c0 + n], ysf[:, c0:c0 + n])
                    nc.vector.tensor_mul(xwf[:, c0:c0 + n], xwf[:, c0:c0 + n], sim[:, :n])
                    out_write(c0, n, xwf, ysf)
                if dbg_keys is not None:
                    nc.sync.dma_start(out=dbg[dbg_keys[0]][:], in_=xk[:])

            def wr_h(c0, n, xwf, ysf):
                nc.vector.tensor_add(xwf[:, c0:c0 + n], xwf[:, c0:c0 + n], ysf[:, c0:c0 + n])

            eaf(hfs, 0, wr_h,
                dbg_keys=("dbg_xk0", "dbg_sim0") if (debug and l == 0) else None)

            xout = x1t if l == 0 else x2t
            for cpad in (0, WP - 1):
                nc.vector.memset(_rap(xout, cpad, [[RC * WP, B], [WP, RC], [1, 1]]), 0.0)
            xof = _flat(xout)

            def wr_l(c0, n, xwf, ysf, xof=xof):
                pos, left = c0, n
                while left > 0:
                    row, col = divmod(pos, W)
                    take = min(left, W - col)
                    nc.vector.tensor_add(xof[:, row * WP + 1 + col: row * WP + 1 + col + take],
                                         xwf[:, pos:pos + take], ysf[:, pos:pos + take])
                    pos += take
                    left -= take

            eaf(lfs, 1, wr_l)
            for (rr, colname) in ((0, "topmask"), (RC - 1, "botmask")):
                ap_ = _rap(xout, rr * WP, [[RC * WP, B], [1, WP]])
                nc.vector.tensor_scalar(ap_, ap_, ppc(colname), None, op0=OP.mult)
            if debug:
                nc.sync.dma_start(out=dbg["dbg_x1" if l == 0 else "dbg_x2"][:], in_=xout[:])

        # ---------------- CAB conv (3x3, 288->96) -> output ----------
        x0b = scr.tile([C, B, RC, WP], BF16, tag="scr")
        nc.sync.dma_start(out=x0b[:], in_=x0s[:])
        wcab = scr.tile([C, 3, 9, C], BF16, tag="scr", name="wcab")
        nc.sync.dma_start(out=wcab[:], in_=w_cab[:])
        cab_chunks = [(i * 2 * WP, 2 * WP - 2) for i in range(RO // 2)]

        def cab_ep(b, c0, n, pt):
            i = c0 // (2 * WP)
            st = at.tile([C, 2 * WP - 2], F32, tag="cabst", name="cabst")
            nc.scalar.activation(st[:], pt, AF.Identity, bias=ppc("cab_b"))
            nc.sync.dma_start(out=out_p[:, b, 2 * i, :], in_=st[:, 0:W])
            nc.sync.dma_start(out=out_p[:, b, 2 * i + 1, :], in_=st[:, WP:WP + W])

        conv3x3([(wcab[:, 0], x0b), (wcab[:, 1], x1t), (wcab[:, 2], x2t)],
                C, cab_chunks, cab_ep)

        stack.close()

    nc.compile()
    _CACHE[key] = nc
    return nc


# ================= host side =================

def _mkw_weight(w, wc, wh, wv, wa, s):
    d = w.shape[0]
    AD_PERM = [3, 0, 1, 6, 4, 2, 7, 8, 5]
    wcf = wc.reshape(d, d, 9).copy()
    wcf[:, :, 4] -= wc.reshape(d, d, 9).sum(-1)
    whd = np.zeros((d, d, 9), np.float32)
    whd[:, :, [0, 3, 6]] = wh
    whd[:, :, [2, 5, 8]] = -wh
    wvd = np.zeros((d, d, 9), np.float32)
    wvd[:, :, 0:3] = wv
    wvd[:, :, 6:9] = -wv
    waf = wa.reshape(d, d, 9)
    wad = waf - waf[:, :, AD_PERM]
    return w + (s[0] * wcf + s[1] * whd + s[2] * wvd + s[3] * wad).reshape(d, d, 3, 3)


def _prep(inputs):
    f32 = np.float32
    g = {k: np.asarray(v, f32) for k, v in inputs.items()}

    wmkw = np.zeros((C, L, 9, C), f32)
    mkw_b = np.zeros((C, L), f32)
    for l in range(L):
        s = g["mkw_scales"][l]
        w = _mkw_weight(g["mkw_w"][l], g["mkw_wc"][l], g["mkw_wh"][l],
                        g["mkw_wv"][l], g["mkw_wa"][l], s)
        wmkw[:, l] = w.reshape(C, C, 9).transpose(1, 2, 0)
        mkw_b[:, l] = (g["mkw_b"][l] + s[0] * g["mkw_bc"][l] + s[1] * g["mkw_bh"][l]
                       + s[2] * g["mkw_bv"][l] + s[3] * g["mkw_ba"][l])

    wqkv = g["dt_wqkv"][:, :, :, 0, 0].transpose(2, 0, 1).copy()
    wdw = g["dt_wdw"][:, :, 0].reshape(L, 3 * C, 9)
    wdwq = np.zeros((C, L, 9, C), f32)
    wdwk = np.zeros((C, L, 9, C), f32)
    wdwv = np.zeros((C, L, 9, 2, 48), f32)
    for ci in range(C):
        wdwq[ci, :, :, ci] = wdw[:, ci]
        wdwk[ci, :, :, ci] = wdw[:, C + ci]
        h, co = divmod(ci, 48)
        wdwv[ci, :, :, h, co] = wdw[:, 2 * C + ci]
    wproj = np.zeros((48, L, 2, C), f32)
    wp_full = g["dt_wproj"][:, :, :, 0, 0]
    for h in range(2):
        wproj[:, :, h, :] = wp_full[:, :, h * 48:(h + 1) * 48].transpose(2, 0, 1)
    weaf = np.zeros((C, L, 2, 2, 48), f32)
    for e, (kx, ky) in enumerate((("eafh_wx", "eafh_wy"), ("eafl_wx", "eafl_wy"))):
        weaf[:, :, e, 0, :] = g[kx][:, :, :, 0, 0].transpose(2, 0, 1)
        weaf[:, :, e, 1, :] = g[ky][:, :, :, 0, 0].transpose(2, 0, 1)
    wcab = g["cab_w"].reshape(C, 3, C, 9).transpose(2, 1, 3, 0).copy()

    def bf(x):
        return np.ascontiguousarray(x.astype(BF))

    def slabs(x, padded):
        wdt = WP if padded else W
        out = []
        for c in range(NCORE):
            sl = np.zeros((C, B, RC, wdt), f32)
            r0 = 24 * c - 1
            for r in range(RC):
                rr = r0 + r
                if 0 <= rr < H:
                    if padded:
                        sl[:, :, r, 1:1 + W] = x[:, :, rr, :].transpose(1, 0, 2)
                    else:
                        sl[:, :, r, :] = x[:, :, rr, :].transpose(1, 0, 2)
            out.append(bf(sl))
        return out

    x0_slabs = slabs(g["x"], True)
    hf_slabs = slabs(g["hf"], False)
    lf_slabs = slabs(g["lf"], False)

    pp_base = np.zeros((C, PPN), f32)
    pp_base[:, PCOL["mkw_b"]:PCOL["mkw_b"] + L] = mkw_b
    for l in range(L):
        for m in range(3):
            pp_base[:, PCOL["qkv_b"] + l * 3 + m] = g["dt_bqkv"][l, m * C:(m + 1) * C]
        pp_base[:, PCOL["dw_bq"] + l] = g["dt_bdw"][l, 0:C]
        pp_base[:, PCOL["dw_bk"] + l] = g["dt_bdw"][l, C:2 * C]
        for h in range(2):
            pp_base[0:48, PCOL["dw_bv"] + l * 2 + h] = \
                g["dt_bdw"][l, 2 * C + h * 48: 2 * C + (h + 1) * 48]
        pp_base[:, PCOL["proj_b"] + l] = g["dt_bproj"][l]
        for p in range(C):
            pp_base[p, PCOL["temp"] + l] = g["dt_temp"][l, p // 48, 0, 0]
        for j in range(4):
            pp_base[:, PCOL["aw"] + l * 4 + j] = g["dt_attnw"][l, j]
    pp_base[:, PCOL["cab_b"]] = g["cab_b"]

    const = dict(
        w_mkw=bf(wmkw), w_qkv=bf(wqkv), w_dwq=bf(wdwq), w_dwk=bf(wdwk),
        w_dwv=bf(wdwv), w_proj=bf(wproj), w_eaf=bf(weaf), w_cab=bf(wcab))
    in_maps = []
    for c in range(NCORE):
        pp2 = pp_base.copy()
        pp2[:, PCOL["topmask"]] = 0.0 if c == 0 else 1.0
        pp2[:, PCOL["botmask"]] = 0.0 if c == NCORE - 1 else 1.0
        in_maps.append(dict(
            x0s=x0_slabs[c], hfs=hf_slabs[c], lfs=lf_slabs[c],
            pp=np.ascontiguousarray(pp2), **const))
    return in_maps


def run(inputs, debug=False):
    nc = _build(debug=debug)
    in_maps = _prep(inputs)
    res = run_bass_kernel_spmd(nc, in_maps, list(range(NCORE)))
    outs = [np.asarray(r["out"], np.float32) for r in res.results]
    full = np.concatenate([o.transpose(1, 0, 2, 3) for o in outs], axis=2)
    return full, res


def kernel(**inputs):
    return run(inputs)[0]


# revision 18
# speedup vs baseline: 10.7993x; 10.7993x over previous
"""Trainium2 Bass kernel for nn_DeepFusionLayers (topk_masking).

Sharding: data-parallel over H (8 cores x 24 rows); halo handled by
recompute from host-prepared overlapping slabs (rows delta in [-1,25)).
3x3 convs = 9 shifted matmuls accumulating in PSUM (C=96 on partitions,
W padded to 194 so shifts never wrap rows). The channel attention runs
on 8x8-maxpooled features: pooled locally, AllGathered (small), then
every core computes the tiny attention redundantly; top-k thresholds via
progressive Max8/match_replace extraction. The 8x nearest upsample is
fused into the proj matmul via step-0 access patterns, reading a
per-core 5-row window of the attention output selected with a dynamic
(partition-id-derived) DRAM slice. EAF BatchNorm uses global stats via
tiny AllGathers; the bn affine transform is folded into the sim matmul
(3 accumulating matmuls over raw conv outputs). bf16 compute, fp32 PSUM.
"""
import numpy as np
import ml_dtypes

import concourse.bass as bass
import concourse.bacc as bacc
import concourse.tile as tile
from concourse import mybir
from concourse.bass_utils import run_bass_kernel_spmd
from concourse.masks import make_identity

F32 = mybir.dt.float32
BF16 = mybir.dt.bfloat16
AF = mybir.ActivationFunctionType
OP = mybir.AluOpType
BF = ml_dtypes.bfloat16

# geometry
B, C, H, W = 2, 96, 192, 192
HEADS, DS, L = 2, 8, 2
NCORE = 8
RO = 24            # owned rows per core
RC = 26            # chain rows: delta in [-1, 25)
WP = W + 2         # padded width (194)
PH, PW = 24, 24    # pooled image
PHL = 3            # pooled rows owned per core
PP2 = PW + 2       # padded pooled width (26)
NP576 = PH * PW
NPOS = B * H * W   # batchnorm population per channel
SENT = -1e30       # topk extraction sentinel
TOPK_KS = [24, 32, 36, 38]
SL3 = (RO - 1) * WP + W  # 3x3 conv flat stream length per sample (4654)

# params column map (f32 tensor "pp" [96, PPN])
_pcols = []
PCOL = {}
def _pc(name, n=1):
    PCOL[name] = len(_pcols)
    _pcols.extend([name] * n)
_pc("mkw_b", L)            # + l
_pc("qkv_b", L * 3)        # + l*3 + m
_pc("dw_bq", L)            # + l
_pc("dw_bk", L)            # + l
_pc("dw_bv", L * 2)        # + l*2 + h  (rows 0..47)
_pc("proj_b", L)           # + l
_pc("cab_b", 1)
_pc("temp", L)             # + l ; row pattern temp[l, p//48]
_pc("aw", L * 4)           # + l*4 + j
_pc("topmask", 1)          # per-core
_pc("botmask", 1)          # per-core
PPN = len(_pcols)

_CACHE = {}


def _chunks(total, maxn=512):
    return [(c, min(maxn, total - c)) for c in range(0, total, maxn)]


def _flat(t):
    ap = t[:]
    nd = len(ap.shape)
    if nd == 2:
        return ap
    names = " ".join(chr(ord("a") + i) for i in range(nd - 1))
    return ap.rearrange(f"p {names} -> p ({names})")


def _rap(t, off, dims):
    """raw AP over pool tile / free-dim offset (elements) + free dims list."""
    return bass.AP(tensor=t.tensor, offset=t.offset + off,
                   ap=[list(t.ap[0])] + [list(d) for d in dims])


def _build(debug=False):
    key = ("mod", debug)
    if key in _CACHE:
        return _CACHE[key]
    nc = bacc.Bacc(None, num_devices=NCORE)

    # ---------------- dram parameters ----------------
    x0s = nc.declare_dram_parameter("x0s", [C, B, RC, WP], BF16, isOutput=False)
    hfs = nc.declare_dram_parameter("hfs", [C, B, RC, W], BF16, isOutput=False)
    lfs = nc.declare_dram_parameter("lfs", [C, B, RC, W], BF16, isOutput=False)
    w_mkw = nc.declare_dram_parameter("w_mkw", [C, L, 9, C], BF16, isOutput=False)
    w_qkv = nc.declare_dram_parameter("w_qkv", [C, L, 3 * C], BF16, isOutput=False)
    w_dwq = nc.declare_dram_parameter("w_dwq", [C, L, 9, C], BF16, isOutput=False)
    w_dwk = nc.declare_dram_parameter("w_dwk", [C, L, 9, C], BF16, isOutput=False)
    w_dwv = nc.declare_dram_parameter("w_dwv", [C, L, 9, 2, 48], BF16, isOutput=False)
    w_proj = nc.declare_dram_parameter("w_proj", [48, L, 2, C], BF16, isOutput=False)
    w_eaf = nc.declare_dram_parameter("w_eaf", [C, L, 2, 2, 48], BF16, isOutput=False)
    w_cab = nc.declare_dram_parameter("w_cab", [C, 3, 9, C], BF16, isOutput=False)
    ppar = nc.declare_dram_parameter("pp", [C, PPN], F32, isOutput=False)
    out_p = nc.declare_dram_parameter("out", [C, B, RO, W], F32, isOutput=True)
    dbg = {}
    if debug:
        def _dbg(name, shape, dt=BF16):
            dbg[name] = nc.declare_dram_parameter(name, shape, dt, isOutput=True)
        _dbg("dbg_mkw0", [C, B, RO, WP])
        _dbg("dbg_pq0", [C, B, PP2, PP2])
        _dbg("dbg_q0", [C, B, NP576])
        _dbg("dbg_v0", [48, B, NP576])
        _dbg("dbg_attn0", [48, 4, 48], F32)
        _dbg("dbg_acomb0", [48, 4, 48], F32)
        _dbg("dbg_os0", [48, 2, B, NP576])
        _dbg("dbg_xdt0", [C, B, RC, W])
        _dbg("dbg_sim0", [C, B, RC, W])
        _dbg("dbg_xk0", [48, B, RC, W])
        _dbg("dbg_x1", [C, B, RC, WP])
        _dbg("dbg_x2", [C, B, RC, WP])

    with tile.TileContext(nc) as tc:
        import contextlib
        stack = contextlib.ExitStack()
        wp = stack.enter_context(tc.tile_pool(name="wp", bufs=1))
        slab = stack.enter_context(tc.tile_pool(name="slab", bufs=1))
        scr = stack.enter_context(tc.tile_pool(name="scr", bufs=3))
        sm = stack.enter_context(tc.tile_pool(name="sm", bufs=1))
        at = stack.enter_context(tc.tile_pool(name="at", bufs=2))
        ps = stack.enter_context(tc.tile_pool(name="ps", bufs=3, space="PSUM"))
        ps2 = stack.enter_context(tc.tile_pool(name="ps2", bufs=3, space="PSUM"))
        dram = stack.enter_context(tc.tile_pool(name="dram", bufs=1, space="DRAM"))

        # ---------------- load weights / params ----------------
        def _load(param, shape, nm, dtype=BF16):
            t = wp.tile(shape, dtype, tag=nm, name=nm)
            nc.sync.dma_start(out=t[:], in_=param[:])
            return t

        wmkw = _load(w_mkw, [C, L, 9, C], "wmkw")
        wqkv = _load(w_qkv, [C, L, 3 * C], "wqkv")
        wdwq = _load(w_dwq, [C, L, 9, C], "wdwq")
        wdwk = _load(w_dwk, [C, L, 9, C], "wdwk")
        wdwv = _load(w_dwv, [C, L, 9, 2, 48], "wdwv")
        wproj = _load(w_proj, [48, L, 2, C], "wproj")
        weaf = _load(w_eaf, [C, L, 2, 2, 48], "weaf")
        pp = _load(ppar, [C, PPN], "ppt", F32)

        def ppc(name, idx=0, p0=0, pn=C):
            c = PCOL[name] + idx
            return pp[p0:p0 + pn, c:c + 1]

        ident = wp.tile([128, 128], BF16)
        make_identity(nc, ident[:])
        ones48 = wp.tile([48, C], BF16)
        nc.vector.memset(ones48[:], 1.0)

        # persistent slabs
        x1t = slab.tile([C, B, RC, WP], BF16, tag="x1t")
        x2t = slab.tile([C, B, RC, WP], BF16, tag="x2t")
        xw = slab.tile([C, B, RC, W], BF16, tag="xw")  # x_dt / x_eh work slab

        # out_small DRAM bounces (pad rows 0 and 25), one per head
        osb = [dram.tile([48, B, PH + 2, PW], BF16, tag=f"osb{h}", name=f"osb{h}") for h in range(HEADS)]
        zpad = sm.tile([48, B, 2, PW], BF16, tag="zpad")
        nc.vector.memset(zpad[:], 0.0)
        for h in range(HEADS):
            nc.sync.dma_start(out=osb[h][:, :, 0:1, :], in_=zpad[:, :, 0:1, :])
            nc.sync.dma_start(out=osb[h][:, :, PH + 1:PH + 2, :], in_=zpad[:, :, 1:2, :])

        # per-core pooled-row window offset register: 3 * partition_id
        with nc.sync.register("goff") as goff:
            nc.sync.reg_load(goff, nc.partition_id_tensor[0:1, 0:1])
            nc.sync.reg_alu(goff, goff, PHL, OP.mult)
            offv = nc.sync.snap(goff)

        rg = [list(range(NCORE))]

        def conv3x3(pairs, M, chunk_list, epilogue):
            """pairs: [(lhsT3 [*,9,C], src_tile [*,B,RC,WP])]; writes per-b
            chunks of the flat output stream (start (row1,col1))."""
            for b in range(B):
                taps = []
                for (lhsT3, src_t) in pairs:
                    sf = _flat(src_t)
                    for t9 in range(9):
                        dy, dx = t9 // 3, t9 % 3
                        off = b * (RC * WP) + dy * WP + dx
                        taps.append(
                            (lhsT3[:, t9, :],
                             (lambda sf, off: lambda c0, n: sf[:, off + c0:off + c0 + n])(sf, off)))
                for (c0, n) in chunk_list:
                    pt = ps.tile([C, 512], F32, tag="cps")
                    for t, (lh, rfn) in enumerate(taps):
                        nc.tensor.matmul(pt[:, :n], lh, rfn(c0, n),
                                         start=(t == 0), stop=(t == len(taps) - 1))
                    epilogue(b, c0, n, pt[:, :n])

        # =====================================================
        for l in range(L):
            if l == 0:
                x0t = scr.tile([C, B, RC, WP], BF16, tag="scr")
                nc.sync.dma_start(out=x0t[:], in_=x0s[:])
                src = x0t
            else:
                src = x1t

            NL = float(B * RO * W)

            def eaf_conv(lhsT, src_t, e_idx, w_i):
                dst = scr.tile([48, B, RC, W], BF16, tag="scr",
                               name=f"ec{e_idx}{w_i}")
                for b in range(B):
                    for r in range(0, RC, 2):
                        pt = ps.tile([48, 512], F32, tag="cps", name="eafpt")
                        nc.tensor.matmul(pt[:48, :384], lhsT, src_t[:, b, r:r + 2, :],
                                         start=True, stop=True)
                        nc.scalar.copy(dst[:, b, r:r + 2, :],
                                       pt[:48, :384].rearrange("p (a b) -> p a b", a=2))
                return dst

            def eaf_stats(src_t, agf, ti, e_idx):
                stt = sm.tile([48, B, 12, 6], F32, tag=f"stt{e_idx}{ti}",
                              name=f"stt{e_idx}{ti}")
                mv = sm.tile([48, 2], F32, tag=f"mv{e_idx}{ti}", name=f"mv{e_idx}{ti}")
                for b in range(B):
                    for i in range(12):
                        nc.vector.bn_stats(
                            out=stt[:, b, i, :],
                            in_=src_t[:, b, 1 + 2 * i:3 + 2 * i, :].rearrange("p a b -> p (a b)"))
                nc.vector.bn_aggr(out=mv[:],
                                  in_=stt[:].rearrange("p a b c -> p (a b) c"))
                nc.vector.tensor_scalar(agf[:, 2 * ti:2 * ti + 1], mv[:, 0:1],
                                        NL, None, op0=OP.mult)
                nc.vector.tensor_mul(agf[:, 2 * ti + 1:2 * ti + 2], mv[:, 0:1], mv[:, 0:1])
                nc.vector.tensor_add(agf[:, 2 * ti + 1:2 * ti + 2],
                                     agf[:, 2 * ti + 1:2 * ti + 2], mv[:, 1:2])
                nc.vector.tensor_scalar(agf[:, 2 * ti + 1:2 * ti + 2],
                                        agf[:, 2 * ti + 1:2 * ti + 2], NL, None, op0=OP.mult)

            def eaf_pre_dma(y_param, e_idx):
                ysl = scr.tile([C, B, RC, W], BF16, tag="scr", name=f"ysl{e_idx}")
                nc.sync.dma_start(out=ysl[:], in_=y_param[:])
                return ysl

            def eaf_pre(y_param, e_idx, ysl=None):
                if ysl is None:
                    ysl = eaf_pre_dma(y_param, e_idx)
                yq = eaf_conv(weaf[:, l, e_idx, 1, :], ysl, e_idx, 1)
                agf = sm.tile([48, 4], F32, tag=f"agf{e_idx}", name=f"agf{e_idx}")
                eaf_stats(yq, agf, 1, e_idx)
                return y_param, yq, agf

            # ---------------- MKW conv (3x3 96->96, owned rows) ---------
            mkwout = scr.tile([C, B, RO, WP], BF16, tag="scr")
            mkf = _flat(mkwout)

            def mkw_ep(b, c0, n, pt):
                nc.scalar.activation(mkf[:, b * (RO * WP) + 1 + c0: b * (RO * WP) + 1 + c0 + n],
                                     pt, AF.Identity, bias=ppc("mkw_b", l))

            conv3x3([(wmkw[:, l], src)], C, _chunks(SL3), mkw_ep)
            if debug and l == 0:
                nc.sync.dma_start(out=dbg["dbg_mkw0"][:], in_=mkwout[:])

            # ---------------- qkv conv (1x1 96->288) + 8x8 maxpool ------
            ag1_in = dram.tile([3, C, B, PHL, PW], BF16)
            ag1_out = dram.tile([3 * NCORE, C, B, PHL, PW], BF16, addr_space="Shared")
            for m in range(3):
                # W-max directly from PSUM (no full-res materialization)
                pwt = sm.tile([C, B, PH, PW], BF16, tag="pwt")
                for b in range(B):
                    for r in range(0, RO, 2):
                        pt = ps.tile([C, 512], F32, tag="cps")
                        nc.tensor.matmul(pt[:, :384], wqkv[:, l, m * C:(m + 1) * C],
                                         mkwout[:, b, r:r + 2, 1:1 + W],
                                         start=True, stop=True)
                        nc.vector.tensor_reduce(
                            out=pwt[:, b, r:r + 2, :],
                            in_=pt[:, :384].rearrange("p (r w e) -> p r w e", r=2, e=DS),
                            axis=mybir.AxisListType.X, op=OP.max)
                pmt = sm.tile([C, B, PHL, PW], BF16, tag="pmt")
                for b in range(B):
                    nc.vector.tensor_reduce(
                        out=pmt[:, b],
                        in_=_rap(pwt, b * PH * PW, [[DS * PW, PHL], [1, PW], [PW, DS]]),
                        axis=mybir.AxisListType.X, op=OP.max)
                contrib = sm.tile([C, B, PHL, PW], BF16, tag="contrib")
                nc.scalar.activation(_flat(contrib), _flat(pmt), AF.Identity,
                                     bias=ppc("qkv_b", l * 3 + m))
                nc.sync.dma_start(out=ag1_in[m], in_=contrib[:])
            nc.gpsimd.collective_compute(
                "AllGather", OP.bypass, replica_groups=rg,
                ins=[ag1_in[:].opt()], outs=[ag1_out[:].opt()])

            pre_h = eaf_pre(hfs, 0)

            # readback into padded pooled tiles (via compact staging)
            pooled = []
            poolc = sm.tile([C, B, PH, PW], BF16, tag="poolc")
            for m in range(3):
                pt_ = sm.tile([C, B, PP2, PP2], BF16, tag=f"pool{m}", name=f"pool{m}")
                nc.vector.memset(pt_[:], 0.0)
                for b in range(B):
                    src_ap = bass.AP(
                        tensor=ag1_out.tensor,
                        offset=ag1_out.offset + m * C * (B * PHL * PW) + b * (PHL * PW),
                        ap=[[B * PHL * PW, C], [3 * C * B * PHL * PW, NCORE],
                            [1, PHL * PW]])
                    dst_ap = _rap(poolc, b * PH * PW, [[PHL * PW, NCORE], [1, PHL * PW]])
                    nc.sync.dma_start(out=dst_ap, in_=src_ap)
                    nc.vector.tensor_copy(
                        _rap(pt_, b * PP2 * PP2 + PP2 + 1, [[PP2, PH], [1, PW]]),
                        _rap(poolc, b * PH * PW, [[PW, PH], [1, PW]]))
                pooled.append(pt_)
            if debug and l == 0:
                nc.sync.dma_start(out=dbg["dbg_pq0"][:], in_=pooled[0][:])

            # ---------------- depthwise 3x3 conv on pooled --------------
            DWCH = [(0, 11 * PP2 + PW), (12 * PP2, 11 * PP2 + PW)]  # rows 1-12, 13-24
            qsb = sm.tile([C, B, NP576], BF16, tag="qsb")
            ksb = sm.tile([C, B, NP576], BF16, tag="ksb")
            vsb = [sm.tile([48, B, NP576], BF16, tag=f"vsb{h}", name=f"vsb{h}") for h in range(HEADS)]

            def dw_run(lhsT9, src_t, M, dst, dst_p0, bias_ap):
                sf = _flat(src_t)
                for b in range(B):
                    for ci, (s0, n) in enumerate(DWCH):
                        pt = ps.tile([M, 512], F32, tag="cps")
                        for t9 in range(9):
                            dy, dx = t9 // 3, t9 % 3
                            off = b * (PP2 * PP2) + dy * PP2 + dx + s0
                            nc.tensor.matmul(pt[:M, :n], lhsT9[:, t9, :],
                                             sf[:, off:off + n],
                                             start=(t9 == 0), stop=(t9 == 8))
                        src_ap = bass.AP(tensor=pt.tensor, offset=pt.offset,
                                         ap=[[pt.ap[0][0], M], [PP2, 12], [1, PW]])
                        dst_ap = bass.AP(
                            tensor=dst.tensor,
                            offset=dst.offset + b * NP576 + ci * 288,
                            ap=[[dst.ap[0][0], M], [PW, 12], [1, PW]])
                        nc.scalar.activation(dst_ap, src_ap, AF.Identity, bias=bias_ap)

            dw_run(wdwq[:, l], pooled[0], C, qsb, 0, ppc("dw_bq", l))
            dw_run(wdwk[:, l], pooled[1], C, ksb, 0, ppc("dw_bk", l))
            for h in range(HEADS):
                dw_run(wdwv[:, l, :, h, :], pooled[2], 48, vsb[h], 0,
                       ppc("dw_bv", l * 2 + h, 0, 48))
            if debug and l == 0:
                nc.sync.dma_start(out=dbg["dbg_q0"][:], in_=qsb[:])
                nc.sync.dma_start(out=dbg["dbg_v0"][:], in_=vsb[0][:])

            # ---------------- normalize q, k (rows of [48,576]) ---------
            sq = sm.tile([C, NP576], F32, tag="sq")
            nrm = sm.tile([C, B, 4], F32, tag="nrm")  # qn kn qscale(kept in 0/1)
            for b in range(B):
                nc.scalar.activation(sq[:], qsb[:, b], AF.Square, accum_out=nrm[:, b, 0:1])
                nc.scalar.activation(sq[:], ksb[:, b], AF.Square, accum_out=nrm[:, b, 1:2])
                nc.scalar.activation(nrm[:, b, 0:2], nrm[:, b, 0:2], AF.Sqrt)
                nc.vector.tensor_scalar_max(nrm[:, b, 0:2], nrm[:, b, 0:2], 1e-12)
                nc.vector.reciprocal(nrm[:, b, 0:2], nrm[:, b, 0:2])
                nc.vector.tensor_scalar(nrm[:, b, 2:3], nrm[:, b, 0:1],
                                        ppc("temp", l), None, op0=OP.mult)
                nc.vector.tensor_scalar(qsb[:, b], qsb[:, b], nrm[:, b, 2:3], None, op0=OP.mult)
                nc.vector.tensor_scalar(ksb[:, b], ksb[:, b], nrm[:, b, 1:2], None, op0=OP.mult)

            # ---------------- transpose q, k ----------------------------
            KCH = [(0, 128), (128, 128), (256, 128), (384, 128), (512, 64)]
            qT = sm.tile([128, B, 5, C], BF16, tag="qT")
            kT = sm.tile([128, B, 5, C], BF16, tag="kT")
            for (src_t, dst) in ((qsb, qT), (ksb, kT)):
                for b in range(B):
                    for ci, (c0, cw) in enumerate(KCH):
                        tp = ps2.tile([128, C], BF16, tag="p2")
                        nc.tensor.transpose(tp[:cw, :], src_t[:, b, c0:c0 + cw],
                                            ident[0:C, 0:C])
                        nc.scalar.copy(dst[:cw, b, ci, :], tp[:cw, :])

            # ---------------- per-pair attention + topk (phase-split) ---
            osm = [at.tile([48, B, NP576], BF16, tag=f"osm{h}", name=f"osm{h}") for h in range(HEADS)]
            PAIRS = [(b, h) for b in range(B) for h in range(HEADS)]
            A_, E_, acc_, wk2_ = {}, {}, {}, {}
            # ph1: attn matmuls + copy + rowmax (PE + DVE)
            rm_ = {}
            for pair, (b, h) in enumerate(PAIRS):
                atp = ps2.tile([48, 48], F32, tag="p2", name="atp")
                for ci, (c0, cw) in enumerate(KCH):
                    nc.tensor.matmul(atp[:], qT[:cw, b, ci, h * 48:(h + 1) * 48],
                                     kT[:cw, b, ci, h * 48:(h + 1) * 48],
                                     start=(ci == 0), stop=(ci == 4))
                A = at.tile([48, 48], F32, tag=f"A{pair}", name=f"A{pair}")
                nc.vector.tensor_copy(A[:], atp[:])
                if debug and l == 0:
                    nc.sync.dma_start(out=dbg["dbg_attn0"][:, pair, :], in_=A[:])
                rm = at.tile([48, 2], F32, tag=f"rm{pair}", name=f"rm{pair}")
                nc.vector.tensor_reduce(out=rm[:, 0:1], in_=A[:],
                                        axis=mybir.AxisListType.X, op=OP.max)
                nc.vector.tensor_scalar_mul(rm[:, 1:2], rm[:, 0:1], -1.0)
                A_[pair], rm_[pair] = A, rm
            # ph2: batched Exp (one ACT table load)
            for pair, (b, h) in enumerate(PAIRS):
                E = at.tile([48, 48], F32, tag=f"E{pair}", name=f"E{pair}")
                nc.scalar.activation(E[:], A_[pair][:], AF.Exp, bias=rm_[pair][:, 1:2])
                E_[pair] = E
            # ph3: per-pair topk chains (DVE only, pairs pipeline)
            for pair, (b, h) in enumerate(PAIRS):
                wk2 = [at.tile([48, 48], F32, tag=f"wka{pair}", name=f"wka{pair}"),
                       at.tile([48, 48], F32, tag=f"wkb{pair}", name=f"wkb{pair}")]
                nc.vector.tensor_copy(wk2[0][:], A_[pair][:])
                cur = 0
                mx = at.tile([48, 8], F32, tag=f"mx{pair}", name=f"mx{pair}")
                acc = at.tile([48, 48], F32, tag=f"acc{pair}", name=f"acc{pair}")
                em = at.tile([48, 48], F32, tag=f"em{pair}", name=f"em{pair}")
                sk = at.tile([48, 2], F32, tag=f"sk{pair}", name=f"sk{pair}")
                wk = at.tile([48, 1], F32, tag=f"wk{pair}", name=f"wk{pair}")
                prev, ik = 0, 0
                for kk in [8, 16, 24, 32, 36, 38]:
                    nfind = kk - prev
                    nc.vector.max(mx[:], wk2[cur][:])
                    if nfind < 8:
                        nc.vector.memset(mx[:, nfind:], SENT)
                    nc.vector.match_replace(out=wk2[1 - cur][:], in_to_replace=mx[:],
                                            in_values=wk2[cur][:], imm_value=SENT)
                    cur = 1 - cur
                    prev = kk
                    if kk in TOPK_KS:
                        nc.vector.tensor_scalar(em[:], wk2[cur][:], SENT, None,
                                                op0=OP.is_equal)
                        nc.vector.tensor_mul(em[:], em[:], E_[pair][:])
                        nc.vector.tensor_reduce(out=sk[:, 0:1], in_=em[:],
                                                axis=mybir.AxisListType.X, op=OP.add)
                        nc.vector.reciprocal(sk[:, 1:2], sk[:, 0:1])
                        nc.vector.tensor_scalar(wk[:], sk[:, 1:2],
                                                ppc("aw", l * 4 + ik, 0, 48),
                                                None, op0=OP.mult)
                        if ik == 0:
                            nc.vector.tensor_scalar(acc[:], em[:], wk[:], None, op0=OP.mult)
                        else:
                            nc.vector.scalar_tensor_tensor(
                                out=acc[:], in0=em[:], scalar=wk[:], in1=acc[:],
                                op0=OP.mult, op1=OP.add)
                        ik += 1
                acc_[pair] = acc
            # ph4: transpose + AV matmuls + batched gelu (2 pair-batches)
            acb_ = {}
            for pair, (b, h) in enumerate(PAIRS):
                acb = at.tile([48, 48], BF16, tag=f"acb{pair}", name=f"acb{pair}")
                nc.vector.tensor_copy(acb[:], acc_[pair][:])
                acb_[pair] = acb
                if debug and l == 0:
                    nc.sync.dma_start(out=dbg["dbg_acomb0"][:, pair, :], in_=acc_[pair][:])
            for pbatch in ([p] for p in PAIRS):
                avp_ = {}
                for (b, h) in pbatch:
                    pair = b * HEADS + h
                    att = ps2.tile([48, 48], BF16, tag="p2", name="att")
                    nc.tensor.transpose(att[:], acb_[pair][:], ident[0:48, 0:48])
                    avw = at.tile([48, 48], BF16, tag=f"avw{pair}", name=f"avw{pair}")
                    nc.scalar.copy(avw[:], att[:])
                    for half in range(2):
                        avp = ps2.tile([48, 288], F32, tag=f"avp{half}",
                                       name=f"avp{pair}_{half}", bufs=1)
                        nc.tensor.matmul(avp[:], avw[:],
                                         vsb[h][:, b, half * 288:(half + 1) * 288],
                                         start=True, stop=True)
                        avp_[(pair, half)] = avp
                for (b, h) in pbatch:
                    pair = b * HEADS + h
                    for half in range(2):
                        nc.scalar.activation(osm[h][:, b, half * 288:(half + 1) * 288],
                                             avp_[(pair, half)][:], AF.Gelu)
            if debug and l == 0:
                for h in range(HEADS):
                    nc.sync.dma_start(out=dbg["dbg_os0"][:, h], in_=osm[h][:])

            # ---------------- window + upsample-fused proj --------------
            for h in range(HEADS):
                nc.sync.dma_start(out=osb[h][:, :, 1:1 + PH, :],
                                  in_=osm[h][:].rearrange("p b (r w) -> p b r w", r=PH))
            wnd = [sm.tile([48, B, 5, PW], BF16, tag=f"wnd{h}", name=f"wnd{h}") for h in range(HEADS)]
            for h in range(HEADS):
                nc.sync.dma_start(out=wnd[h][:], in_=osb[h][:, :, bass.ds(offv, 5), :])
            for b in range(B):
                for r in range(RC):
                    wr = (r - 1) // DS + 1
                    pt = ps.tile([C, 512], F32, tag="cps")
                    for h in range(HEADS):
                        rhs = bass.AP(
                            tensor=wnd[h].tensor,
                            offset=wnd[h].offset + (b * 5 + wr) * PW,
                            ap=[[wnd[h].ap[0][0], 48], [1, PW], [0, DS]])
                        nc.tensor.matmul(pt[:, :W], wproj[:, l, h, :], rhs,
                                         start=(h == 0), stop=(h == 1))
                    nc.scalar.activation(xw[:, b, r, :], pt[:, :W], AF.Identity,
                                         bias=ppc("proj_b", l))
            if debug and l == 0:
                nc.sync.dma_start(out=dbg["dbg_xdt0"][:], in_=xw[:])

            # ---------------- EAF (h then l) ----------------------------

            def eaf(pre, e_idx, out_write, fill_fn=None, dbg_keys=None):
                y_param, yq, agf = pre
                yflat = y_param[:].rearrange("p a b c -> p (a b c)")
                xk = eaf_conv(weaf[:, l, e_idx, 0, :], xw, e_idx, 0)
                eaf_stats(xk, agf, 0, e_idx)
                ags_in = dram.tile([48, 4], F32)
                ags_out = dram.tile([48 * NCORE, 4], F32, addr_space="Shared")
                nc.sync.dma_start(out=ags_in[:], in_=agf[:])
                nc.gpsimd.collective_compute(
                    "AllGather", OP.bypass, replica_groups=rg,
                    ins=[ags_in[:].opt()], outs=[ags_out[:].opt()])
                fill = fill_fn() if fill_fn is not None else None
                rb = sm.tile([48, 4, NCORE], F32, tag="rb")
                nc.sync.dma_start(out=rb[:], in_=bass.AP(
                    tensor=ags_out.tensor, offset=ags_out.offset,
                    ap=[[4, 48], [1, 4], [48 * 4, NCORE]]))
                gs = sm.tile([48, 4], F32, tag="gs")
                nc.vector.tensor_reduce(out=gs[:], in_=rb[:],
                                        axis=mybir.AxisListType.X, op=OP.add)
                # s,t per channel: mean=S/N; var=Q/N-mean^2; s=1/sqrt(var+eps); t=-mean*s
                stv = sm.tile([48, 8], F32, tag="stv")  # sx tx sy ty | ss st ts tt
                for ti in range(2):
                    nc.vector.tensor_scalar(stv[:, 4:5], gs[:, 2 * ti:2 * ti + 1],
                                            1.0 / NPOS, None, op0=OP.mult)
                    nc.vector.tensor_scalar(stv[:, 5:6], gs[:, 2 * ti + 1:2 * ti + 2],
                                            1.0 / NPOS, None, op0=OP.mult)
                    nc.vector.tensor_mul(stv[:, 6:7], stv[:, 4:5], stv[:, 4:5])
                    nc.vector.tensor_sub(stv[:, 5:6], stv[:, 5:6], stv[:, 6:7])
                    nc.vector.tensor_scalar(stv[:, 5:6], stv[:, 5:6], 1e-5, None, op0=OP.add)
                    nc.scalar.activation(stv[:, 5:6], stv[:, 5:6], AF.Sqrt)
                    nc.vector.reciprocal(stv[:, 2 * ti:2 * ti + 1], stv[:, 5:6])
                    nc.vector.tensor_mul(stv[:, 2 * ti + 1:2 * ti + 2], stv[:, 4:5],
                                         stv[:, 2 * ti:2 * ti + 1])
                    nc.vector.tensor_scalar_mul(stv[:, 2 * ti + 1:2 * ti + 2],
                                                stv[:, 2 * ti + 1:2 * ti + 2], -1.0)
                nc.vector.tensor_mul(stv[:, 4:5], stv[:, 0:1], stv[:, 2:3])  # ss
                nc.vector.tensor_mul(stv[:, 5:6], stv[:, 0:1], stv[:, 3:4])  # st
                nc.vector.tensor_mul(stv[:, 6:7], stv[:, 1:2], stv[:, 2:3])  # ts
                nc.vector.tensor_mul(stv[:, 7:8], stv[:, 1:2], stv[:, 3:4])  # tt
                vss = sm.tile([48, C], BF16, tag="vss")
                vst = sm.tile([48, C], BF16, tag="vst")
                vts = sm.tile([48, C], BF16, tag="vts")
                nc.vector.tensor_scalar(vss[:], ones48[:], stv[:, 4:5], None, op0=OP.mult)
                nc.vector.tensor_scalar(vst[:], ones48[:], stv[:, 5:6], None, op0=OP.mult)
                nc.vector.tensor_scalar(vts[:], ones48[:], stv[:, 6:7], None, op0=OP.mult)
                vtt = sm.tile([48, 1], BF16, tag="vtt")
                nc.vector.tensor_copy(vtt[:], stv[:, 7:8])
                cb = ps2.tile([C, 1], F32, tag="p2")
                nc.tensor.matmul(cb[:], ones48[:], vtt[:], start=True, stop=True)
                cbs = sm.tile([C, 1], F32, tag="cbs")
                nc.vector.tensor_copy(cbs[:], cb[:])
                # sim + blend, chunked (y streamed from DRAM per chunk)
                xkf, yqf, xwf = _flat(xk), _flat(yq), _flat(xw)
                for (c0, n) in _chunks(B * RC * W):
                    ych = sm.tile([C, 512], BF16, tag=f"ych{e_idx}", name=f"ych{e_idx}", bufs=3)
                    nc.sync.dma_start(out=ych[:, :n], in_=yflat[:, c0:c0 + n])
                    pch = sm.tile([48, 512], BF16, tag="pch", bufs=3)
                    nc.vector.tensor_mul(pch[:, :n], xkf[:, c0:c0 + n], yqf[:, c0:c0 + n])
                    spt = ps.tile([C, 512], F32, tag="cps")
                    nc.tensor.matmul(spt[:, :n], vss[:], pch[:48, :n], start=True, stop=False)
                    nc.tensor.matmul(spt[:, :n], vst[:], xkf[:, c0:c0 + n], start=False, stop=False)
                    nc.tensor.matmul(spt[:, :n], vts[:], yqf[:, c0:c0 + n], start=False, stop=True)
                    sim = sm.tile([C, 512], BF16, tag="sim", bufs=3)
                    nc.scalar.activation(sim[:, :n], spt[:, :n], AF.Sigmoid, bias=cbs[:, 0:1])
                    if dbg_keys is not None:
                        nc.sync.dma_start(out=_flat(dbg[dbg_keys[1]])[:, c0:c0 + n],
                                          in_=sim[:, :n])
                    nc.vector.tensor_sub(xwf[:, c0:c0 + n], xwf[:, c0:c0 + n], ych[:, :n])
                    nc.vector.tensor_mul(xwf[:, c0:c0 + n], xwf[:, c0:c0 + n], sim[:, :n])
                    out_write(c0, n, xwf, ych)
                if dbg_keys is not None:
                    nc.sync.dma_start(out=dbg[dbg_keys[0]][:], in_=xk[:])

            def wr_h(c0, n, xwf, ych):
                nc.vector.tensor_add(xwf[:, c0:c0 + n], xwf[:, c0:c0 + n], ych[:, :n])

            box = {}

            def fill_pre_l():
                box["ysl1"] = eaf_pre_dma(lfs, 1)

            eaf(pre_h, 0, wr_h, fill_fn=fill_pre_l,
                dbg_keys=("dbg_xk0", "dbg_sim0") if (debug and l == 0) else None)
            pre_l = eaf_pre(lfs, 1, ysl=box["ysl1"])

            xout = x1t if l == 0 else x2t
            for cpad in (0, WP - 1):
                nc.vector.memset(_rap(xout, cpad, [[RC * WP, B], [WP, RC], [1, 1]]), 0.0)
            xof = _flat(xout)

            def wr_l(c0, n, xwf, ych, xof=xof):
                pos, left = c0, n
                while left > 0:
                    row, col = divmod(pos, W)
                    take = min(left, W - col)
                    nc.vector.tensor_add(xof[:, row * WP + 1 + col: row * WP + 1 + col + take],
                                         xwf[:, pos:pos + take], ych[:, pos - c0:pos - c0 + take])
                    pos += take
                    left -= take

            eaf(pre_l, 1, wr_l)
            for (rr, colname) in ((0, "topmask"), (RC - 1, "botmask")):
                ap_ = _rap(xout, rr * WP, [[RC * WP, B], [1, WP]])
                nc.vector.tensor_scalar(ap_, ap_, ppc(colname), None, op0=OP.mult)
            if debug:
                nc.sync.dma_start(out=dbg["dbg_x1" if l == 0 else "dbg_x2"][:], in_=xout[:])

        # ---------------- CAB conv (3x3, 288->96) -> output ----------
        x0b = scr.tile([C, B, RC, WP], BF16, tag="scr")
        nc.sync.dma_start(out=x0b[:], in_=x0s[:])
        wcab = scr.tile([C, 3, 9, C], BF16, tag="scr", name="wcab")
        nc.sync.dma_start(out=wcab[:], in_=w_cab[:])
        cab_chunks = [(i * 2 * WP, 2 * WP - 2) for i in range(RO // 2)]

        def cab_ep(b, c0, n, pt):
            i = c0 // (2 * WP)
            st = at.tile([C, 2 * WP - 2], F32, tag="cabst", name="cabst")
            nc.scalar.activation(st[:], pt, AF.Identity, bias=ppc("cab_b"))
            nc.sync.dma_start(out=out_p[:, b, 2 * i, :], in_=st[:, 0:W])
            nc.sync.dma_start(out=out_p[:, b, 2 * i + 1, :], in_=st[:, WP:WP + W])

        conv3x3([(wcab[:, 0], x0b), (wcab[:, 1], x1t), (wcab[:, 2], x2t)],
                C, cab_chunks, cab_ep)

        stack.close()

    nc.compile()
    _CACHE[key] = nc
    return nc


# ================= host side =================

def _mkw_weight(w, wc, wh, wv, wa, s):
    d = w.shape[0]
    AD_PERM = [3, 0, 1, 6, 4, 2, 7, 8, 5]
    wcf = wc.reshape(d, d, 9).copy()
    wcf[:, :, 4] -= wc.reshape(d, d, 9).sum(-1)
    whd = np.zeros((d, d, 9), np.float32)
    whd[:, :, [0, 3, 6]] = wh
    whd[:, :, [2, 5, 8]] = -wh
    wvd = np.zeros((d, d, 9), np.float32)
    wvd[:, :, 0:3] = wv
    wvd[:, :, 6:9] = -wv
    waf = wa.reshape(d, d, 9)
    wad = waf - waf[:, :, AD_PERM]
    return w + (s[0] * wcf + s[1] * whd + s[2] * wvd + s[3] * wad).reshape(d, d, 3, 3)


def _prep(inputs):
    f32 = np.float32
    g = {k: np.asarray(v, f32) for k, v in inputs.items()}

    wmkw = np.zeros((C, L, 9, C), f32)
    mkw_b = np.zeros((C, L), f32)
    for l in range(L):
        s = g["mkw_scales"][l]
        w = _mkw_weight(g["mkw_w"][l], g["mkw_wc"][l], g["mkw_wh"][l],
                        g["mkw_wv"][l], g["mkw_wa"][l], s)
        wmkw[:, l] = w.reshape(C, C, 9).transpose(1, 2, 0)
        mkw_b[:, l] = (g["mkw_b"][l] + s[0] * g["mkw_bc"][l] + s[1] * g["mkw_bh"][l]
                       + s[2] * g["mkw_bv"][l] + s[3] * g["mkw_ba"][l])

    wqkv = g["dt_wqkv"][:, :, :, 0, 0].transpose(2, 0, 1).copy()
    wdw = g["dt_wdw"][:, :, 0].reshape(L, 3 * C, 9)
    wdwq = np.zeros((C, L, 9, C), f32)
    wdwk = np.zeros((C, L, 9, C), f32)
    wdwv = np.zeros((C, L, 9, 2, 48), f32)
    for ci in range(C):
        wdwq[ci, :, :, ci] = wdw[:, ci]
        wdwk[ci, :, :, ci] = wdw[:, C + ci]
        h, co = divmod(ci, 48)
        wdwv[ci, :, :, h, co] = wdw[:, 2 * C + ci]
    wproj = np.zeros((48, L, 2, C), f32)
    wp_full = g["dt_wproj"][:, :, :, 0, 0]
    for h in range(2):
        wproj[:, :, h, :] = wp_full[:, :, h * 48:(h + 1) * 48].transpose(2, 0, 1)
    weaf = np.zeros((C, L, 2, 2, 48), f32)
    for e, (kx, ky) in enumerate((("eafh_wx", "eafh_wy"), ("eafl_wx", "eafl_wy"))):
        weaf[:, :, e, 0, :] = g[kx][:, :, :, 0, 0].transpose(2, 0, 1)
        weaf[:, :, e, 1, :] = g[ky][:, :, :, 0, 0].transpose(2, 0, 1)
    wcab = g["cab_w"].reshape(C, 3, C, 9).transpose(2, 1, 3, 0).copy()

    def bf(x):
        return np.ascontiguousarray(x.astype(BF))

    def slabs(x, padded):
        wdt = WP if padded else W
        out = []
        for c in range(NCORE):
            sl = np.zeros((C, B, RC, wdt), f32)
            r0 = 24 * c - 1
            for r in range(RC):
                rr = r0 + r
                if 0 <= rr < H:
                    if padded:
                        sl[:, :, r, 1:1 + W] = x[:, :, rr, :].transpose(1, 0, 2)
                    else:
                        sl[:, :, r, :] = x[:, :, rr, :].transpose(1, 0, 2)
            out.append(bf(sl))
        return out

    x0_slabs = slabs(g["x"], True)
    hf_slabs = slabs(g["hf"], False)
    lf_slabs = slabs(g["lf"], False)

    pp_base = np.zeros((C, PPN), f32)
    pp_base[:, PCOL["mkw_b"]:PCOL["mkw_b"] + L] = mkw_b
    for l in range(L):
        for m in range(3):
            pp_base[:, PCOL["qkv_b"] + l * 3 + m] = g["dt_bqkv"][l, m * C:(m + 1) * C]
        pp_base[:, PCOL["dw_bq"] + l] = g["dt_bdw"][l, 0:C]
        pp_base[:, PCOL["dw_bk"] + l] = g["dt_bdw"][l, C:2 * C]
        for h in range(2):
            pp_base[0:48, PCOL["dw_bv"] + l * 2 + h] = \
                g["dt_bdw"][l, 2 * C + h * 48: 2 * C + (h + 1) * 48]
        pp_base[:, PCOL["proj_b"] + l] = g["dt_bproj"][l]
        for p in range(C):
            pp_base[p, PCOL["temp"] + l] = g["dt_temp"][l, p // 48, 0, 0]
        for j in range(4):
            pp_base[:, PCOL["aw"] + l * 4 + j] = g["dt_attnw"][l, j]
    pp_base[:, PCOL["cab_b"]] = g["cab_b"]

    const = dict(
        w_mkw=bf(wmkw), w_qkv=bf(wqkv), w_dwq=bf(wdwq), w_dwk=bf(wdwk),
        w_dwv=bf(wdwv), w_proj=bf(wproj), w_eaf=bf(weaf), w_cab=bf(wcab))
    in_maps = []
    for c in range(NCORE):
        pp2 = pp_base.copy()
        pp2[:, PCOL["topmask"]] = 0.0 if c == 0 else 1.0
        pp2[:, PCOL["botmask"]] = 0.0 if c == NCORE - 1 else 1.0
        in_maps.append(dict(
            x0s=x0_slabs[c], hfs=hf_slabs[c], lfs=lf_slabs[c],
            pp=np.ascontiguousarray(pp2), **const))
    return in_maps


def run(inputs, debug=False):
    nc = _build(debug=debug)
    in_maps = _prep(inputs)
    res = run_bass_kernel_spmd(nc, in_maps, list(range(NCORE)))
    outs = [np.asarray(r["out"], np.float32) for r in res.results]
    full = np.concatenate([o.transpose(1, 0, 2, 3) for o in outs], axis=2)
    return full, res


def kernel(**inputs):
    return run(inputs)[0]


# revision 21
# speedup vs baseline: 18.0649x; 1.6728x over previous
"""Trainium2 Bass kernel for nn_DeepFusionLayers (topk_masking).

Sharding: data-parallel over H (8 cores x 24 rows); halo handled by
recompute from host-prepared overlapping slabs (rows delta in [-1,25)).
3x3 convs = 9 shifted matmuls accumulating in PSUM (C=96 on partitions,
W padded to 194 so shifts never wrap rows). The channel attention runs
on 8x8-maxpooled features: pooled locally, AllGathered (small), then
every core computes the tiny attention redundantly; top-k thresholds via
progressive Max8/match_replace extraction. The 8x nearest upsample is
fused into the proj matmul via step-0 access patterns, reading a
per-core 5-row window of the attention output selected with a dynamic
(partition-id-derived) DRAM slice. EAF BatchNorm uses global stats via
tiny AllGathers; the bn affine transform is folded into the sim matmul
(3 accumulating matmuls over raw conv outputs). bf16 compute, fp32 PSUM.
"""
import numpy as np
import ml_dtypes

import concourse.bass as bass
import concourse.bacc as bacc
import concourse.tile as tile
from concourse import mybir
from concourse.bass_utils import run_bass_kernel_spmd
from concourse.masks import make_identity

F32 = mybir.dt.float32
BF16 = mybir.dt.bfloat16
AF = mybir.ActivationFunctionType
OP = mybir.AluOpType
BF = ml_dtypes.bfloat16

# geometry
B, C, H, W = 2, 96, 192, 192
HEADS, DS, L = 2, 8, 2
NCORE = 8
RO = 24            # owned rows per core
RC = 26            # chain rows: delta in [-1, 25)
WP = W + 2         # padded width (194)
PH, PW = 24, 24    # pooled image
PHL = 3            # pooled rows owned per core
PP2 = PW + 2       # padded pooled width (26)
NP576 = PH * PW
NPOS = B * H * W   # batchnorm population per channel
SENT = -1e30       # topk extraction sentinel
TOPK_KS = [24, 32, 36, 38]
SL3 = (RO - 1) * WP + W  # 3x3 conv flat stream length per sample (4654)

# params column map (f32 tensor "pp" [96, PPN])
_pcols = []
PCOL = {}
def _pc(name, n=1):
    PCOL[name] = len(_pcols)
    _pcols.extend([name] * n)
_pc("mkw_b", L)            # + l
_pc("qkv_b", L * 3)        # + l*3 + m
_pc("dw_bq", L)            # + l
_pc("dw_bk", L)            # + l
_pc("dw_bv", L * 2)        # + l*2 + h  (rows 0..47)
_pc("proj_b", L)           # + l
_pc("cab_b", 1)
_pc("temp", L)             # + l ; row pattern temp[l, p//48]
_pc("aw", L * 4)           # + l*4 + j
_pc("topmask", 1)          # per-core
_pc("botmask", 1)          # per-core
PPN = len(_pcols)

_CACHE = {}


def _chunks(total, maxn=512):
    return [(c, min(maxn, total - c)) for c in range(0, total, maxn)]


def _flat(t):
    ap = t[:]
    nd = len(ap.shape)
    if nd == 2:
        return ap
    names = " ".join(chr(ord("a") + i) for i in range(nd - 1))
    return ap.rearrange(f"p {names} -> p ({names})")


def _rap(t, off, dims):
    """raw AP over pool tile / free-dim offset (elements) + free dims list."""
    return bass.AP(tensor=t.tensor, offset=t.offset + off,
                   ap=[list(t.ap[0])] + [list(d) for d in dims])


def _build(debug=False):
    key = ("mod", debug)
    if key in _CACHE:
        return _CACHE[key]
    nc = bacc.Bacc(None, num_devices=NCORE)

    # ---------------- dram parameters ----------------
    x0s = nc.declare_dram_parameter("x0s", [C, B, RC, WP], BF16, isOutput=False)
    hfs = nc.declare_dram_parameter("hfs", [C, B, RC, W], BF16, isOutput=False)
    lfs = nc.declare_dram_parameter("lfs", [C, B, RC, W], BF16, isOutput=False)
    w_mkw = nc.declare_dram_parameter("w_mkw", [C, L, 9, C], BF16, isOutput=False)
    w_qkv = nc.declare_dram_parameter("w_qkv", [C, L, 3 * C], BF16, isOutput=False)
    w_dwq = nc.declare_dram_parameter("w_dwq", [C, L, 9, C], BF16, isOutput=False)
    w_dwk = nc.declare_dram_parameter("w_dwk", [C, L, 9, C], BF16, isOutput=False)
    w_dwv = nc.declare_dram_parameter("w_dwv", [C, L, 9, 2, 48], BF16, isOutput=False)
    w_proj = nc.declare_dram_parameter("w_proj", [48, L, 2, C], BF16, isOutput=False)
    w_eaf = nc.declare_dram_parameter("w_eaf", [C, L, 2, 2, 48], BF16, isOutput=False)
    w_cab = nc.declare_dram_parameter("w_cab", [C, 3, 9, C], BF16, isOutput=False)
    ppar = nc.declare_dram_parameter("pp", [C, PPN], F32, isOutput=False)
    out_p = nc.declare_dram_parameter("out", [C, B, RO, W], F32, isOutput=True)
    dbg = {}
    if debug:
        def _dbg(name, shape, dt=BF16):
            dbg[name] = nc.declare_dram_parameter(name, shape, dt, isOutput=True)
        _dbg("dbg_mkw0", [C, B, RO, WP])
        _dbg("dbg_pq0", [C, B, PP2, PP2])
        _dbg("dbg_q0", [C, B, NP576])
        _dbg("dbg_v0", [48, B, NP576])
        _dbg("dbg_attn0", [48, 4, 48], F32)
        _dbg("dbg_acomb0", [48, 4, 48], F32)
        _dbg("dbg_os0", [48, 2, B, NP576])
        _dbg("dbg_xdt0", [C, B, RC, W])
        _dbg("dbg_sim0", [C, B, RC, W])
        _dbg("dbg_xk0", [48, B, RC, W])
        _dbg("dbg_x1", [C, B, RC, WP])
        _dbg("dbg_x2", [C, B, RC, WP])

    with tile.TileContext(nc) as tc:
        import contextlib
        stack = contextlib.ExitStack()
        wp = stack.enter_context(tc.tile_pool(name="wp", bufs=1))
        slab = stack.enter_context(tc.tile_pool(name="slab", bufs=1))
        scr = stack.enter_context(tc.tile_pool(name="scr", bufs=3))
        sm = stack.enter_context(tc.tile_pool(name="sm", bufs=1))
        at = stack.enter_context(tc.tile_pool(name="at", bufs=2))
        ps = stack.enter_context(tc.tile_pool(name="ps", bufs=3, space="PSUM"))
        ps2 = stack.enter_context(tc.tile_pool(name="ps2", bufs=3, space="PSUM"))
        dram = stack.enter_context(tc.tile_pool(name="dram", bufs=1, space="DRAM"))

        # ---------------- load weights / params ----------------
        def _load(param, shape, nm, dtype=BF16):
            t = wp.tile(shape, dtype, tag=nm, name=nm)
            nc.sync.dma_start(out=t[:], in_=param[:])
            return t

        wmkw = _load(w_mkw, [C, L, 9, C], "wmkw")
        wqkv = _load(w_qkv, [C, L, 3 * C], "wqkv")
        wdwq = _load(w_dwq, [C, L, 9, C], "wdwq")
        wdwk = _load(w_dwk, [C, L, 9, C], "wdwk")
        wdwv = _load(w_dwv, [C, L, 9, 2, 48], "wdwv")
        wproj = _load(w_proj, [48, L, 2, C], "wproj")
        weaf = _load(w_eaf, [C, L, 2, 2, 48], "weaf")
        pp = _load(ppar, [C, PPN], "ppt", F32)

        def ppc(name, idx=0, p0=0, pn=C):
            c = PCOL[name] + idx
            return pp[p0:p0 + pn, c:c + 1]

        ident = wp.tile([128, 128], BF16)
        make_identity(nc, ident[:])
        ones48 = wp.tile([48, C], BF16)
        nc.vector.memset(ones48[:], 1.0)

        # persistent slabs
        x1t = slab.tile([C, B, RC, WP], BF16, tag="x1t")
        x2t = slab.tile([C, B, RC, WP], BF16, tag="x2t")
        xw = slab.tile([C, B, RC, W], BF16, tag="xw")  # x_dt / x_eh work slab

        # out_small DRAM bounces (pad rows 0 and 25), one per head
        osb = [dram.tile([48, B, PH + 2, PW], BF16, tag=f"osb{h}", name=f"osb{h}") for h in range(HEADS)]
        zpad = sm.tile([48, B, 2, PW], BF16, tag="zpad")
        nc.vector.memset(zpad[:], 0.0)
        for h in range(HEADS):
            nc.sync.dma_start(out=osb[h][:, :, 0:1, :], in_=zpad[:, :, 0:1, :])
            nc.sync.dma_start(out=osb[h][:, :, PH + 1:PH + 2, :], in_=zpad[:, :, 1:2, :])

        # per-core pooled-row window offset register: 3 * partition_id
        with nc.sync.register("goff") as goff:
            nc.sync.reg_load(goff, nc.partition_id_tensor[0:1, 0:1])
            nc.sync.reg_alu(goff, goff, PHL, OP.mult)
            offv = nc.sync.snap(goff)

        rg = [list(range(NCORE))]

        def conv3x3(pairs, M, chunk_list, epilogue):
            """pairs: [(lhsT3 [*,9,C], src_tile [*,B,RC,WP])]; writes per-b
            chunks of the flat output stream (start (row1,col1))."""
            for b in range(B):
                taps = []
                for (lhsT3, src_t) in pairs:
                    sf = _flat(src_t)
                    for t9 in range(9):
                        dy, dx = t9 // 3, t9 % 3
                        off = b * (RC * WP) + dy * WP + dx
                        taps.append(
                            (lhsT3[:, t9, :],
                             (lambda sf, off: lambda c0, n: sf[:, off + c0:off + c0 + n])(sf, off)))
                for (c0, n) in chunk_list:
                    pt = ps.tile([C, 512], F32, tag="cps")
                    for t, (lh, rfn) in enumerate(taps):
                        nc.tensor.matmul(pt[:, :n], lh, rfn(c0, n),
                                         start=(t == 0), stop=(t == len(taps) - 1))
                    epilogue(b, c0, n, pt[:, :n])

        # =====================================================
        for l in range(L):
            if l == 0:
                x0t = scr.tile([C, B, RC, WP], BF16, tag="scr")
                for b_ in range(B):
                    nc.sync.dma_start(out=x0t[:, b_], in_=x0s[:, b_])
                src = x0t
            else:
                src = x1t

            NL = float(B * RO * W)

            def eaf_conv(lhsT, src_t, e_idx, w_i):
                dst = scr.tile([48, B, RC, W], BF16, tag="scr",
                               name=f"ec{e_idx}{w_i}")
                for b in range(B):
                    for r in range(0, RC, 2):
                        pt = ps.tile([48, 512], F32, tag="cps", name="eafpt")
                        nc.tensor.matmul(pt[:48, :384], lhsT, src_t[:, b, r:r + 2, :],
                                         start=True, stop=True)
                        nc.scalar.copy(dst[:, b, r:r + 2, :],
                                       pt[:48, :384].rearrange("p (a b) -> p a b", a=2))
                return dst

            def eaf_stats(src_t, agf, ti, e_idx):
                stt = sm.tile([48, B, 12, 6], F32, tag=f"stt{e_idx}{ti}",
                              name=f"stt{e_idx}{ti}")
                mv = sm.tile([48, 2], F32, tag=f"mv{e_idx}{ti}", name=f"mv{e_idx}{ti}")
                for b in range(B):
                    for i in range(12):
                        nc.vector.bn_stats(
                            out=stt[:, b, i, :],
                            in_=src_t[:, b, 1 + 2 * i:3 + 2 * i, :].rearrange("p a b -> p (a b)"))
                nc.vector.bn_aggr(out=mv[:],
                                  in_=stt[:].rearrange("p a b c -> p (a b) c"))
                nc.vector.tensor_scalar(agf[:, 2 * ti:2 * ti + 1], mv[:, 0:1],
                                        NL, None, op0=OP.mult)
                nc.vector.tensor_mul(agf[:, 2 * ti + 1:2 * ti + 2], mv[:, 0:1], mv[:, 0:1])
                nc.vector.tensor_add(agf[:, 2 * ti + 1:2 * ti + 2],
                                     agf[:, 2 * ti + 1:2 * ti + 2], mv[:, 1:2])
                nc.vector.tensor_scalar(agf[:, 2 * ti + 1:2 * ti + 2],
                                        agf[:, 2 * ti + 1:2 * ti + 2], NL, None, op0=OP.mult)

            def eaf_pre_dma(y_param, e_idx):
                ysl = scr.tile([C, B, RC, W], BF16, tag="scr", name=f"ysl{e_idx}")
                nc.sync.dma_start(out=ysl[:], in_=y_param[:])
                return ysl

            def eaf_pre(y_param, e_idx, ysl=None):
                if ysl is None:
                    ysl = eaf_pre_dma(y_param, e_idx)
                yq = eaf_conv(weaf[:, l, e_idx, 1, :], ysl, e_idx, 1)
                agf = sm.tile([48, 4], F32, tag=f"agf{e_idx}", name=f"agf{e_idx}")
                eaf_stats(yq, agf, 1, e_idx)
                return y_param, yq, agf

            # ---------------- MKW conv (3x3 96->96, owned rows) ---------
            mkwout = scr.tile([C, B, RO, WP], BF16, tag="scr")
            mkf = _flat(mkwout)

            def mkw_ep(b, c0, n, pt):
                nc.scalar.activation(mkf[:, b * (RO * WP) + 1 + c0: b * (RO * WP) + 1 + c0 + n],
                                     pt, AF.Identity, bias=ppc("mkw_b", l))

            conv3x3([(wmkw[:, l], src)], C, _chunks(SL3), mkw_ep)
            if debug and l == 0:
                nc.sync.dma_start(out=dbg["dbg_mkw0"][:], in_=mkwout[:])

            # ---------------- qkv conv (1x1 96->288) + 8x8 maxpool ------
            # per-m AllGathers: AG(m) latency overlaps m+1's conv/pooling,
            # and the dw conv of m starts as soon as its own AG lands.
            ag_ins = [dram.tile([C, B, PHL, PW], BF16, tag=f"agin{m}", name=f"agin{m}")
                      for m in range(3)]
            ag_outs = [dram.tile([NCORE * C, B, PHL, PW], BF16, addr_space="Shared",
                                 tag=f"agout{m}", name=f"agout{m}") for m in range(3)]
            for m in range(3):
                # W-max directly from PSUM (no full-res materialization)
                pwt = sm.tile([C, B, PH, PW], BF16, tag="pwt", bufs=2)
                for b in range(B):
                    for r in range(0, RO, 2):
                        pt = ps.tile([C, 512], F32, tag="cps")
                        nc.tensor.matmul(pt[:, :384], wqkv[:, l, m * C:(m + 1) * C],
                                         mkwout[:, b, r:r + 2, 1:1 + W],
                                         start=True, stop=True)
                        nc.vector.tensor_reduce(
                            out=pwt[:, b, r:r + 2, :],
                            in_=pt[:, :384].rearrange("p (r w e) -> p r w e", r=2, e=DS),
                            axis=mybir.AxisListType.X, op=OP.max)
                pmt = sm.tile([C, B, PHL, PW], BF16, tag="pmt", bufs=2)
                for b in range(B):
                    nc.vector.tensor_reduce(
                        out=pmt[:, b],
                        in_=_rap(pwt, b * PH * PW, [[DS * PW, PHL], [1, PW], [PW, DS]]),
                        axis=mybir.AxisListType.X, op=OP.max)
                contrib = sm.tile([C, B, PHL, PW], BF16, tag="contrib", bufs=2)
                nc.scalar.activation(_flat(contrib), _flat(pmt), AF.Identity,
                                     bias=ppc("qkv_b", l * 3 + m))
                nc.sync.dma_start(out=ag_ins[m][:], in_=contrib[:])
                nc.gpsimd.collective_compute(
                    "AllGather", OP.bypass, replica_groups=rg,
                    ins=[ag_ins[m][:].opt()], outs=[ag_outs[m][:].opt()])
            pre_h = eaf_pre(hfs, 0)

            # readback into padded pooled tiles (via compact staging)
            pooled = []
            poolc = sm.tile([C, B, PH, PW], BF16, tag="poolc")
            for m in range(3):
                pt_ = sm.tile([C, B, PP2, PP2], BF16, tag=f"pool{m}", name=f"pool{m}")
                nc.vector.memset(pt_[:], 0.0)
                for b in range(B):
                    src_ap = bass.AP(
                        tensor=ag_outs[m].tensor,
                        offset=ag_outs[m].offset + b * (PHL * PW),
                        ap=[[B * PHL * PW, C], [C * B * PHL * PW, NCORE],
                            [1, PHL * PW]])
                    dst_ap = _rap(poolc, b * PH * PW, [[PHL * PW, NCORE], [1, PHL * PW]])
                    nc.sync.dma_start(out=dst_ap, in_=src_ap)
                    nc.vector.tensor_copy(
                        _rap(pt_, b * PP2 * PP2 + PP2 + 1, [[PP2, PH], [1, PW]]),
                        _rap(poolc, b * PH * PW, [[PW, PH], [1, PW]]))
                pooled.append(pt_)
            if debug and l == 0:
                nc.sync.dma_start(out=dbg["dbg_pq0"][:], in_=pooled[0][:])

            # ---------------- depthwise 3x3 conv on pooled --------------
            DWCH = [(0, 11 * PP2 + PW), (12 * PP2, 11 * PP2 + PW)]  # rows 1-12, 13-24
            qsb = sm.tile([C, B, NP576], BF16, tag="qsb")
            ksb = sm.tile([C, B, NP576], BF16, tag="ksb")
            vsb = [sm.tile([48, B, NP576], BF16, tag=f"vsb{h}", name=f"vsb{h}") for h in range(HEADS)]

            def dw_run(lhsT9, src_t, M, dst, dst_p0, bias_ap):
                sf = _flat(src_t)
                for b in range(B):
                    for ci, (s0, n) in enumerate(DWCH):
                        pt = ps.tile([M, 512], F32, tag="cps")
                        for t9 in range(9):
                            dy, dx = t9 // 3, t9 % 3
                            off = b * (PP2 * PP2) + dy * PP2 + dx + s0
                            nc.tensor.matmul(pt[:M, :n], lhsT9[:, t9, :],
                                             sf[:, off:off + n],
                                             start=(t9 == 0), stop=(t9 == 8))
                        src_ap = bass.AP(tensor=pt.tensor, offset=pt.offset,
                                         ap=[[pt.ap[0][0], M], [PP2, 12], [1, PW]])
                        dst_ap = bass.AP(
                            tensor=dst.tensor,
                            offset=dst.offset + b * NP576 + ci * 288,
                            ap=[[dst.ap[0][0], M], [PW, 12], [1, PW]])
                        nc.scalar.activation(dst_ap, src_ap, AF.Identity, bias=bias_ap)

            dw_run(wdwq[:, l], pooled[0], C, qsb, 0, ppc("dw_bq", l))
            dw_run(wdwk[:, l], pooled[1], C, ksb, 0, ppc("dw_bk", l))
            for h in range(HEADS):
                dw_run(wdwv[:, l, :, h, :], pooled[2], 48, vsb[h], 0,
                       ppc("dw_bv", l * 2 + h, 0, 48))
            if debug and l == 0:
                nc.sync.dma_start(out=dbg["dbg_q0"][:], in_=qsb[:])
                nc.sync.dma_start(out=dbg["dbg_v0"][:], in_=vsb[0][:])

            # ---------------- normalize q, k (rows of [48,576]) ---------
            sq = sm.tile([C, NP576], F32, tag="sq")
            nrm = sm.tile([C, B, 4], F32, tag="nrm")  # qn kn qscale(kept in 0/1)
            for b in range(B):
                nc.scalar.activation(sq[:], qsb[:, b], AF.Square, accum_out=nrm[:, b, 0:1])
                nc.scalar.activation(sq[:], ksb[:, b], AF.Square, accum_out=nrm[:, b, 1:2])
                nc.scalar.activation(nrm[:, b, 0:2], nrm[:, b, 0:2], AF.Sqrt)
                nc.vector.tensor_scalar_max(nrm[:, b, 0:2], nrm[:, b, 0:2], 1e-12)
                nc.vector.reciprocal(nrm[:, b, 0:2], nrm[:, b, 0:2])
                nc.vector.tensor_scalar(nrm[:, b, 2:3], nrm[:, b, 0:1],
                                        ppc("temp", l), None, op0=OP.mult)
                nc.vector.tensor_scalar(qsb[:, b], qsb[:, b], nrm[:, b, 2:3], None, op0=OP.mult)
                nc.vector.tensor_scalar(ksb[:, b], ksb[:, b], nrm[:, b, 1:2], None, op0=OP.mult)

            # ---------------- transpose q, k ----------------------------
            KCH = [(0, 128), (128, 128), (256, 128), (384, 128), (512, 64)]
            qT = sm.tile([128, B, 5, C], BF16, tag="qT")
            kT = sm.tile([128, B, 5, C], BF16, tag="kT")
            for (src_t, dst) in ((qsb, qT), (ksb, kT)):
                for b in range(B):
                    for ci, (c0, cw) in enumerate(KCH):
                        tp = ps2.tile([128, C], BF16, tag="p2")
                        nc.tensor.transpose(tp[:cw, :], src_t[:, b, c0:c0 + cw],
                                            ident[0:C, 0:C])
                        nc.scalar.copy(dst[:cw, b, ci, :], tp[:cw, :])

            # ---------------- per-pair attention + topk (phase-split) ---
            osm = [at.tile([48, B, NP576], BF16, tag=f"osm{h}", name=f"osm{h}") for h in range(HEADS)]
            PAIRS = [(b, h) for b in range(B) for h in range(HEADS)]
            A_, E_, acc_, wk2_ = {}, {}, {}, {}
            # ph1: attn matmuls + copy + rowmax (PE + DVE)
            rm_ = {}
            for pair, (b, h) in enumerate(PAIRS):
                atp = ps2.tile([48, 48], F32, tag="p2", name="atp")
                for ci, (c0, cw) in enumerate(KCH):
                    nc.tensor.matmul(atp[:], qT[:cw, b, ci, h * 48:(h + 1) * 48],
                                     kT[:cw, b, ci, h * 48:(h + 1) * 48],
                                     start=(ci == 0), stop=(ci == 4))
                A = at.tile([48, 48], F32, tag=f"A{pair}", name=f"A{pair}")
                nc.vector.tensor_copy(A[:], atp[:])
                if debug and l == 0:
                    nc.sync.dma_start(out=dbg["dbg_attn0"][:, pair, :], in_=A[:])
                rm = at.tile([48, 2], F32, tag=f"rm{pair}", name=f"rm{pair}")
                nc.vector.tensor_reduce(out=rm[:, 0:1], in_=A[:],
                                        axis=mybir.AxisListType.X, op=OP.max)
                nc.vector.tensor_scalar_mul(rm[:, 1:2], rm[:, 0:1], -1.0)
                A_[pair], rm_[pair] = A, rm
            # ph2: batched Exp (one ACT table load)
            for pair, (b, h) in enumerate(PAIRS):
                E = at.tile([48, 48], F32, tag=f"E{pair}", name=f"E{pair}")
                nc.scalar.activation(E[:], A_[pair][:], AF.Exp, bias=rm_[pair][:, 1:2])
                E_[pair] = E
            # ph3: per-pair topk chains (DVE only, pairs pipeline)
            for pair, (b, h) in enumerate(PAIRS):
                wk2 = [at.tile([48, 48], F32, tag=f"wka{pair}", name=f"wka{pair}"),
                       at.tile([48, 48], F32, tag=f"wkb{pair}", name=f"wkb{pair}")]
                nc.vector.tensor_copy(wk2[0][:], A_[pair][:])
                cur = 0
                mx = at.tile([48, 8], F32, tag=f"mx{pair}", name=f"mx{pair}")
                acc = at.tile([48, 48], F32, tag=f"acc{pair}", name=f"acc{pair}")
                em = at.tile([48, 48], F32, tag=f"em{pair}", name=f"em{pair}")
                sk = at.tile([48, 2], F32, tag=f"sk{pair}", name=f"sk{pair}")
                wk = at.tile([48, 1], F32, tag=f"wk{pair}", name=f"wk{pair}")
                prev, ik = 0, 0
                for kk in [8, 16, 24, 32, 36, 38]:
                    nfind = kk - prev
                    nc.vector.max(mx[:], wk2[cur][:])
                    if nfind < 8:
                        nc.vector.memset(mx[:, nfind:], SENT)
                    nc.vector.match_replace(out=wk2[1 - cur][:], in_to_replace=mx[:],
                                            in_values=wk2[cur][:], imm_value=SENT)
                    cur = 1 - cur
                    prev = kk
                    if kk in TOPK_KS:
                        nc.vector.scalar_tensor_tensor(
                            out=em[:], in0=wk2[cur][:], scalar=SENT, in1=E_[pair][:],
                            op0=OP.is_equal, op1=OP.mult)
                        nc.vector.tensor_reduce(out=sk[:, 0:1], in_=em[:],
                                                axis=mybir.AxisListType.X, op=OP.add)
                        nc.vector.reciprocal(sk[:, 1:2], sk[:, 0:1])
                        nc.vector.tensor_scalar(wk[:], sk[:, 1:2],
                                                ppc("aw", l * 4 + ik, 0, 48),
                                                None, op0=OP.mult)
                        if ik == 0:
                            nc.vector.tensor_scalar(acc[:], em[:], wk[:], None, op0=OP.mult)
                        else:
                            nc.vector.scalar_tensor_tensor(
                                out=acc[:], in0=em[:], scalar=wk[:], in1=acc[:],
                                op0=OP.mult, op1=OP.add)
                        ik += 1
                acc_[pair] = acc
            # ph4: transpose + AV matmuls + batched gelu (2 pair-batches)
            acb_ = {}
            for pair, (b, h) in enumerate(PAIRS):
                acb = at.tile([48, 48], BF16, tag=f"acb{pair}", name=f"acb{pair}")
                nc.vector.tensor_copy(acb[:], acc_[pair][:])
                acb_[pair] = acb
                if debug and l == 0:
                    nc.sync.dma_start(out=dbg["dbg_acomb0"][:, pair, :], in_=acc_[pair][:])
            for pbatch in ([p] for p in PAIRS):
                avp_ = {}
                for (b, h) in pbatch:
                    pair = b * HEADS + h
                    att = ps2.tile([48, 48], BF16, tag="p2", name="att")
                    nc.tensor.transpose(att[:], acb_[pair][:], ident[0:48, 0:48])
                    avw = at.tile([48, 48], BF16, tag=f"avw{pair}", name=f"avw{pair}")
                    nc.scalar.copy(avw[:], att[:])
                    for half in range(2):
                        avp = ps2.tile([48, 288], F32, tag=f"avp{half}",
                                       name=f"avp{pair}_{half}", bufs=1)
                        nc.tensor.matmul(avp[:], avw[:],
                                         vsb[h][:, b, half * 288:(half + 1) * 288],
                                         start=True, stop=True)
                        avp_[(pair, half)] = avp
                for (b, h) in pbatch:
                    pair = b * HEADS + h
                    for half in range(2):
                        nc.scalar.activation(osm[h][:, b, half * 288:(half + 1) * 288],
                                             avp_[(pair, half)][:], AF.Gelu)
            if debug and l == 0:
                for h in range(HEADS):
                    nc.sync.dma_start(out=dbg["dbg_os0"][:, h], in_=osm[h][:])

            # ---------------- window + upsample-fused proj --------------
            for h in range(HEADS):
                nc.sync.dma_start(out=osb[h][:, :, 1:1 + PH, :],
                                  in_=osm[h][:].rearrange("p b (r w) -> p b r w", r=PH))
            wnd = [sm.tile([48, B, 5, PW], BF16, tag=f"wnd{h}", name=f"wnd{h}") for h in range(HEADS)]
            for h in range(HEADS):
                nc.sync.dma_start(out=wnd[h][:], in_=osb[h][:, :, bass.ds(offv, 5), :])
            PRCH = [(0, 1)] + [(1 + 2 * i, 2) for i in range(12)] + [(RC - 1, 1)]
            for b in range(B):
                for (r0, nr) in PRCH:
                    wr = (r0 - 1) // DS + 1
                    pt = ps.tile([C, 512], F32, tag="cps")
                    for h in range(HEADS):
                        rhs = bass.AP(
                            tensor=wnd[h].tensor,
                            offset=wnd[h].offset + (b * 5 + wr) * PW,
                            ap=[[wnd[h].ap[0][0], 48], [0, nr], [1, PW], [0, DS]])
                        nc.tensor.matmul(pt[:, :nr * W], wproj[:, l, h, :], rhs,
                                         start=(h == 0), stop=(h == 1))
                    nc.scalar.activation(
                        xw[:, b, r0:r0 + nr, :].rearrange("p a b -> p (a b)"),
                        pt[:, :nr * W], AF.Identity, bias=ppc("proj_b", l))
            if debug and l == 0:
                nc.sync.dma_start(out=dbg["dbg_xdt0"][:], in_=xw[:])

            # ---------------- EAF (h then l) ----------------------------

            def eaf(pre, e_idx, out_write, fill_fn=None, dbg_keys=None):
                y_param, yq, agf = pre
                yflat = y_param[:].rearrange("p a b c -> p (a b c)")
                xk = eaf_conv(weaf[:, l, e_idx, 0, :], xw, e_idx, 0)
                eaf_stats(xk, agf, 0, e_idx)
                ags_in = dram.tile([48, 4], F32)
                ags_out = dram.tile([48 * NCORE, 4], F32, addr_space="Shared")
                nc.sync.dma_start(out=ags_in[:], in_=agf[:])
                nc.gpsimd.collective_compute(
                    "AllGather", OP.bypass, replica_groups=rg,
                    ins=[ags_in[:].opt()], outs=[ags_out[:].opt()])
                fill = fill_fn() if fill_fn is not None else None
                rb = sm.tile([48, 4, NCORE], F32, tag="rb")
                nc.sync.dma_start(out=rb[:], in_=bass.AP(
                    tensor=ags_out.tensor, offset=ags_out.offset,
                    ap=[[4, 48], [1, 4], [48 * 4, NCORE]]))
                gs = sm.tile([48, 4], F32, tag="gs")
                nc.vector.tensor_reduce(out=gs[:], in_=rb[:],
                                        axis=mybir.AxisListType.X, op=OP.add)
                # s,t per channel: mean=S/N; var=Q/N-mean^2; s=1/sqrt(var+eps); t=-mean*s
                stv = sm.tile([48, 8], F32, tag="stv")  # sx tx sy ty | ss st ts tt
                for ti in range(2):
                    nc.vector.tensor_scalar(stv[:, 4:5], gs[:, 2 * ti:2 * ti + 1],
                                            1.0 / NPOS, None, op0=OP.mult)
                    nc.vector.tensor_scalar(stv[:, 5:6], gs[:, 2 * ti + 1:2 * ti + 2],
                                            1.0 / NPOS, None, op0=OP.mult)
                    nc.vector.tensor_mul(stv[:, 6:7], stv[:, 4:5], stv[:, 4:5])
                    nc.vector.tensor_sub(stv[:, 5:6], stv[:, 5:6], stv[:, 6:7])
                    nc.vector.tensor_scalar(stv[:, 5:6], stv[:, 5:6], 1e-5, None, op0=OP.add)
                    nc.scalar.activation(stv[:, 5:6], stv[:, 5:6], AF.Sqrt)
                    nc.vector.reciprocal(stv[:, 2 * ti:2 * ti + 1], stv[:, 5:6])
                    nc.vector.tensor_mul(stv[:, 2 * ti + 1:2 * ti + 2], stv[:, 4:5],
                                         stv[:, 2 * ti:2 * ti + 1])
                    nc.vector.tensor_scalar_mul(stv[:, 2 * ti + 1:2 * ti + 2],
                                                stv[:, 2 * ti + 1:2 * ti + 2], -1.0)
                nc.vector.tensor_mul(stv[:, 4:5], stv[:, 0:1], stv[:, 2:3])  # ss
                nc.vector.tensor_mul(stv[:, 5:6], stv[:, 0:1], stv[:, 3:4])  # st
                nc.vector.tensor_mul(stv[:, 6:7], stv[:, 1:2], stv[:, 2:3])  # ts
                nc.vector.tensor_mul(stv[:, 7:8], stv[:, 1:2], stv[:, 3:4])  # tt
                vss = sm.tile([48, C], BF16, tag="vss")
                vst = sm.tile([48, C], BF16, tag="vst")
                vts = sm.tile([48, C], BF16, tag="vts")
                nc.vector.tensor_scalar(vss[:], ones48[:], stv[:, 4:5], None, op0=OP.mult)
                nc.vector.tensor_scalar(vst[:], ones48[:], stv[:, 5:6], None, op0=OP.mult)
                nc.vector.tensor_scalar(vts[:], ones48[:], stv[:, 6:7], None, op0=OP.mult)
                vtt = sm.tile([48, 1], BF16, tag="vtt")
                nc.vector.tensor_copy(vtt[:], stv[:, 7:8])
                cb = ps2.tile([C, 1], F32, tag="p2")
                nc.tensor.matmul(cb[:], ones48[:], vtt[:], start=True, stop=True)
                cbs = sm.tile([C, 1], F32, tag="cbs")
                nc.vector.tensor_copy(cbs[:], cb[:])
                # sim + blend, chunked (y streamed from DRAM per chunk)
                xkf, yqf, xwf = _flat(xk), _flat(yq), _flat(xw)
                for (c0, n) in _chunks(B * RC * W):
                    ych = sm.tile([C, 512], BF16, tag=f"ych{e_idx}", name=f"ych{e_idx}", bufs=3)
                    nc.sync.dma_start(out=ych[:, :n], in_=yflat[:, c0:c0 + n])
                    pch = sm.tile([48, 512], BF16, tag="pch", bufs=3)
                    nc.vector.tensor_mul(pch[:, :n], xkf[:, c0:c0 + n], yqf[:, c0:c0 + n])
                    spt = ps.tile([C, 512], F32, tag="cps")
                    nc.tensor.matmul(spt[:, :n], vss[:], pch[:48, :n], start=True, stop=False)
                    nc.tensor.matmul(spt[:, :n], vst[:], xkf[:, c0:c0 + n], start=False, stop=False)
                    nc.tensor.matmul(spt[:, :n], vts[:], yqf[:, c0:c0 + n], start=False, stop=True)
                    sim = sm.tile([C, 512], BF16, tag="sim", bufs=3)
                    nc.scalar.activation(sim[:, :n], spt[:, :n], AF.Sigmoid, bias=cbs[:, 0:1])
                    if dbg_keys is not None:
                        nc.sync.dma_start(out=_flat(dbg[dbg_keys[1]])[:, c0:c0 + n],
                                          in_=sim[:, :n])
                    nc.vector.tensor_sub(xwf[:, c0:c0 + n], xwf[:, c0:c0 + n], ych[:, :n])
                    nc.vector.tensor_mul(xwf[:, c0:c0 + n], xwf[:, c0:c0 + n], sim[:, :n])
                    out_write(c0, n, xwf, ych)
                if dbg_keys is not None:
                    nc.sync.dma_start(out=dbg[dbg_keys[0]][:], in_=xk[:])

            def wr_h(c0, n, xwf, ych):
                nc.vector.tensor_add(xwf[:, c0:c0 + n], xwf[:, c0:c0 + n], ych[:, :n])

            box = {}

            def fill_pre_l():
                box["ysl1"] = eaf_pre_dma(lfs, 1)

            eaf(pre_h, 0, wr_h, fill_fn=fill_pre_l,
                dbg_keys=("dbg_xk0", "dbg_sim0") if (debug and l == 0) else None)
            pre_l = eaf_pre(lfs, 1, ysl=box["ysl1"])

            xout = x1t if l == 0 else x2t
            for cpad in (0, WP - 1):
                nc.vector.memset(_rap(xout, cpad, [[RC * WP, B], [WP, RC], [1, 1]]), 0.0)
            xof = _flat(xout)

            def wr_l(c0, n, xwf, ych, xof=xof):
                pos, left = c0, n
                while left > 0:
                    row, col = divmod(pos, W)
                    take = min(left, W - col)
                    nc.vector.tensor_add(xof[:, row * WP + 1 + col: row * WP + 1 + col + take],
                                         xwf[:, pos:pos + take], ych[:, pos - c0:pos - c0 + take])
                    pos += take
                    left -= take

            eaf(pre_l, 1, wr_l)
            for (rr, colname) in ((0, "topmask"), (RC - 1, "botmask")):
                ap_ = _rap(xout, rr * WP, [[RC * WP, B], [1, WP]])
                nc.vector.tensor_scalar(ap_, ap_, ppc(colname), None, op0=OP.mult)
            if debug:
                nc.sync.dma_start(out=dbg["dbg_x1" if l == 0 else "dbg_x2"][:], in_=xout[:])

        # ---------------- CAB conv (3x3, 288->96) -> output ----------
        x0b = scr.tile([C, B, RC, WP], BF16, tag="scr")
        nc.sync.dma_start(out=x0b[:], in_=x0s[:])
        wcab = scr.tile([C, 3, 9, C], BF16, tag="scr", name="wcab")
        nc.sync.dma_start(out=wcab[:], in_=w_cab[:])
        cab_chunks = [(i * 2 * WP, 2 * WP - 2) for i in range(RO // 2)]

        def cab_ep(b, c0, n, pt):
            i = c0 // (2 * WP)
            st = at.tile([C, 2 * WP - 2], F32, tag="cabst", name="cabst")
            nc.scalar.activation(st[:], pt, AF.Identity, bias=ppc("cab_b"))
            nc.sync.dma_start(out=out_p[:, b, 2 * i, :], in_=st[:, 0:W])
            nc.sync.dma_start(out=out_p[:, b, 2 * i + 1, :], in_=st[:, WP:WP + W])

        conv3x3([(wcab[:, 0], x0b), (wcab[:, 1], x1t), (wcab[:, 2], x2t)],
                C, cab_chunks, cab_ep)

        stack.close()

    nc.compile()
    _CACHE[key] = nc
    return nc


# ================= host side =================

def _mkw_weight(w, wc, wh, wv, wa, s):
    d = w.shape[0]
    AD_PERM = [3, 0, 1, 6, 4, 2, 7, 8, 5]
    wcf = wc.reshape(d, d, 9).copy()
    wcf[:, :, 4] -= wc.reshape(d, d, 9).sum(-1)
    whd = np.zeros((d, d, 9), np.float32)
    whd[:, :, [0, 3, 6]] = wh
    whd[:, :, [2, 5, 8]] = -wh
    wvd = np.zeros((d, d, 9), np.float32)
    wvd[:, :, 0:3] = wv
    wvd[:, :, 6:9] = -wv
    waf = wa.reshape(d, d, 9)
    wad = waf - waf[:, :, AD_PERM]
    return w + (s[0] * wcf + s[1] * whd + s[2] * wvd + s[3] * wad).reshape(d, d, 3, 3)


def _prep(inputs):
    f32 = np.float32
    g = {k: np.asarray(v, f32) for k, v in inputs.items()}

    wmkw = np.zeros((C, L, 9, C), f32)
    mkw_b = np.zeros((C, L), f32)
    for l in range(L):
        s = g["mkw_scales"][l]
        w = _mkw_weight(g["mkw_w"][l], g["mkw_wc"][l], g["mkw_wh"][l],
                        g["mkw_wv"][l], g["mkw_wa"][l], s)
        wmkw[:, l] = w.reshape(C, C, 9).transpose(1, 2, 0)
        mkw_b[:, l] = (g["mkw_b"][l] + s[0] * g["mkw_bc"][l] + s[1] * g["mkw_bh"][l]
                       + s[2] * g["mkw_bv"][l] + s[3] * g["mkw_ba"][l])

    wqkv = g["dt_wqkv"][:, :, :, 0, 0].transpose(2, 0, 1).copy()
    wdw = g["dt_wdw"][:, :, 0].reshape(L, 3 * C, 9)
    wdwq = np.zeros((C, L, 9, C), f32)
    wdwk = np.zeros((C, L, 9, C), f32)
    wdwv = np.zeros((C, L, 9, 2, 48), f32)
    for ci in range(C):
        wdwq[ci, :, :, ci] = wdw[:, ci]
        wdwk[ci, :, :, ci] = wdw[:, C + ci]
        h, co = divmod(ci, 48)
        wdwv[ci, :, :, h, co] = wdw[:, 2 * C + ci]
    wproj = np.zeros((48, L, 2, C), f32)
    wp_full = g["dt_wproj"][:, :, :, 0, 0]
    for h in range(2):
        wproj[:, :, h, :] = wp_full[:, :, h * 48:(h + 1) * 48].transpose(2, 0, 1)
    weaf = np.zeros((C, L, 2, 2, 48), f32)
    for e, (kx, ky) in enumerate((("eafh_wx", "eafh_wy"), ("eafl_wx", "eafl_wy"))):
        weaf[:, :, e, 0, :] = g[kx][:, :, :, 0, 0].transpose(2, 0, 1)
        weaf[:, :, e, 1, :] = g[ky][:, :, :, 0, 0].transpose(2, 0, 1)
    wcab = g["cab_w"].reshape(C, 3, C, 9).transpose(2, 1, 3, 0).copy()

    def bf(x):
        return np.ascontiguousarray(x.astype(BF))

    def slabs(x, padded):
        wdt = WP if padded else W
        out = []
        for c in range(NCORE):
            sl = np.zeros((C, B, RC, wdt), f32)
            r0 = 24 * c - 1
            for r in range(RC):
                rr = r0 + r
                if 0 <= rr < H:
                    if padded:
                        sl[:, :, r, 1:1 + W] = x[:, :, rr, :].transpose(1, 0, 2)
                    else:
                        sl[:, :, r, :] = x[:, :, rr, :].transpose(1, 0, 2)
            out.append(bf(sl))
        return out

    x0_slabs = slabs(g["x"], True)
    hf_slabs = slabs(g["hf"], False)
    lf_slabs = slabs(g["lf"], False)

    pp_base = np.zeros((C, PPN), f32)
    pp_base[:, PCOL["mkw_b"]:PCOL["mkw_b"] + L] = mkw_b
    for l in range(L):
        for m in range(3):
            pp_base[:, PCOL["qkv_b"] + l * 3 + m] = g["dt_bqkv"][l, m * C:(m + 1) * C]
        pp_base[:, PCOL["dw_bq"] + l] = g["dt_bdw"][l, 0:C]
        pp_base[:, PCOL["dw_bk"] + l] = g["dt_bdw"][l, C:2 * C]
        for h in range(2):
            pp_base[0:48, PCOL["dw_bv"] + l * 2 + h] = \
                g["dt_bdw"][l, 2 * C + h * 48: 2 * C + (h + 1) * 48]
        pp_base[:, PCOL["proj_b"] + l] = g["dt_bproj"][l]
        for p in range(C):
            pp_base[p, PCOL["temp"] + l] = g["dt_temp"][l, p // 48, 0, 0]
        for j in range(4):
            pp_base[:, PCOL["aw"] + l * 4 + j] = g["dt_attnw"][l, j]
    pp_base[:, PCOL["cab_b"]] = g["cab_b"]

    const = dict(
        w_mkw=bf(wmkw), w_qkv=bf(wqkv), w_dwq=bf(wdwq), w_dwk=bf(wdwk),
        w_dwv=bf(wdwv), w_proj=bf(wproj), w_eaf=bf(weaf), w_cab=bf(wcab))
    in_maps = []
    for c in range(NCORE):
        pp2 = pp_base.copy()
        pp2[:, PCOL["topmask"]] = 0.0 if c == 0 else 1.0
        pp2[:, PCOL["botmask"]] = 0.0 if c == NCORE - 1 else 1.0
        in_maps.append(dict(
            x0s=x0_slabs[c], hfs=hf_slabs[c], lfs=lf_slabs[c],
            pp=np.ascontiguousarray(pp2), **const))
    return in_maps


def run(inputs, debug=False):
    nc = _build(debug=debug)
    in_maps = _prep(inputs)
    res = run_bass_kernel_spmd(nc, in_maps, list(range(NCORE)))
    outs = [np.asarray(r["out"], np.float32) for r in res.results]
    full = np.concatenate([o.transpose(1, 0, 2, 3) for o in outs], axis=2)
    return full, res


def kernel(**inputs):
    return run(inputs)[0]
